# revision 30
# baseline (speedup 1.0000x reference)
"""Sliding-window GQA attention (Gemma-style) on 8 TRN2 NeuronCores.

Sharding: tensor-parallel over heads. Core c owns q-heads {2c, 2c+1} and
kv-head c. Each core computes Q/K/V projections (+RoPE) for its heads over
the full sequence, banded sliding-window attention, then an AllToAll
(split into 2 token-chunks) reshards the attention output by token so
every core computes the full output projection for its 512-token slice.
Host concatenates slices.

Attention uses a [k, q] logits layout (K stationary, Q moving) so the
softmax probabilities leave the Scalar engine directly in the [k, q]
orientation PV needs -- no PE transposes. QK logits for two adjacent key
tiles share one PSUM bank so one Tanh activation drains them straight
into a wide SBUF strip; one Exp activation per band segment produces
bf16 probabilities. Window/causal masks are added by GpSimd. Softmax
denominators come from ones-vector matmuls accumulated in PSUM and the
normalization is applied once on the (much smaller) attention output.

Batch-0 attention is interleaved into batch-1's projection so the
Scalar/Vector-engine softmax work hides under projection matmuls.

All matmuls run in bf16 (f32 PSUM accumulation); softmax runs in f32.
"""

import contextlib
import os
import sys

for _p in ("/opt/trn_rl_repo",):
    if _p not in sys.path:
        sys.path.insert(0, _p)

import numpy as np
import ml_dtypes

import concourse.bass as bass
import concourse.mybir as mybir
import concourse.tile as tile
from concourse import bacc
from concourse.bass_utils import run_bass_kernel_spmd

F32 = mybir.dt.float32
BF16 = mybir.dt.bfloat16
AF = mybir.ActivationFunctionType
ALU = mybir.AluOpType

B, T, D = 2, 2048, 3584
NQ, NKV, H = 16, 8, 256
SCALAR = 0.0625
SOFT_CAP = 50.0
WINDOW = 1024
ROPE_BASE = 10000.0

NCORES = 8
P = 128
DC = D // P              # 28 contraction chunks
TQ = T // P              # 16 query tiles per batch
TPIECE = 256             # projection output tile width
NPIECE = T // TPIECE
HLOC = 2 * H             # 512 local q-head columns per core
TOK = B * T              # 4096
TPC = TOK // NCORES      # 512 tokens per core after AllToAll
CHTOK = TPC // 2         # 256 tokens per A2A chunk block
WTILES = WINDOW // P     # 8
MASKVAL = -1.0e30        # added to tanh output; exp(50*(t+MASKVAL)) == 0
NHC = NQ * H // P        # 32 global h chunks
DP = 512                 # output projection d piece
SEGJ = 9                 # max band tiles per softmax strip

last_result = None       # BassKernelResults of the most recent device run


def _band(i, mode):
    lo = max(0, i - WTILES)
    hi = i if mode == "tril" else min(TQ - 1, i + WTILES)
    return lo, hi


def build(mode):
    assert mode in ("tril", "ones")
    nc = bacc.Bacc("TRN2", target_bir_lowering=False, debug=False,
                   num_devices=NCORES)

    xT = nc.dram_tensor("xT", [D, TOK], BF16, kind="ExternalInput")
    wq = nc.dram_tensor("wq", [D, HLOC], BF16, kind="ExternalInput")
    wk = nc.dram_tensor("wk", [D, H], BF16, kind="ExternalInput")
    wv = nc.dram_tensor("wv", [D, H], BF16, kind="ExternalInput")
    wo = nc.dram_tensor("wo", [NHC, P, D], BF16, kind="ExternalInput")
    ropeq = nc.dram_tensor("ropeq", [2, P, T], BF16, kind="ExternalInput")
    ropek = nc.dram_tensor("ropek", [2, P, T], BF16, kind="ExternalInput")
    msk = nc.dram_tensor("msk", [3, P, 2 * P], F32, kind="ExternalInput")
    out = nc.dram_tensor("out", [TPC, D], F32, kind="ExternalOutput")

    with tile.TileContext(nc) as tc:
        with (
            tc.tile_pool(name="dram", bufs=1, space="DRAM") as dram,
            tc.tile_pool(name="consts", bufs=1) as consts,
            tc.tile_pool(name="qkv", bufs=1) as qkvpool,
            tc.tile_pool(name="attn_sb", bufs=2) as apool,
            tc.tile_pool(name="es_sb", bufs=3) as espool,
            tc.tile_pool(name="rb_sb", bufs=1) as rbpool,
        ):
            # A2A bounce buffers: [src_rank_block][local h chunk-major][tok]
            a2a_in = [dram.tile([NCORES, HLOC, CHTOK], BF16,
                                name=f"a2a_in{m}") for m in range(2)]
            a2a_out = [dram.tile([NCORES, HLOC, CHTOK], BF16,
                                 name=f"a2a_out{m}") for m in range(2)]

            ones_col = consts.tile([P, 1], BF16)
            nc.gpsimd.memset(ones_col, 1.0)
            msk_sb = consts.tile([P, 3, 2 * P], F32)

            qts, kts, vsbs = [], [], []
            P_POOLS = {}
            xT_v = xT[:].rearrange("(c p) t -> p c t", p=P)

            # weights/x pools close mid-schedule (after projections) so
            # their SBUF space is reused by the wo/ef pools.
            pstack = contextlib.ExitStack()
            wpool = pstack.enter_context(
                tc.tile_pool(name="weights", bufs=1))
            xpool = pstack.enter_context(tc.tile_pool(name="xt", bufs=2))
            rpool = pstack.enter_context(tc.tile_pool(name="rtmp", bufs=1))

            # ---- staged input loads: first slices small so the first
            # matmuls start within a few us of kernel start ----
            wq_sb = wpool.tile([P, DC, HLOC], BF16)
            wq_v = wq[:].rearrange("(c p) h -> p c h", p=P)
            nc.sync.dma_start(wq_sb[:, 0:4, :], wq_v[:, 0:4, :])
            xt00 = xpool.tile([P, DC, TPIECE], BF16, tag="xt")
            nc.sync.dma_start(xt00[:, 0:4, :], xT_v[:, 0:4, 0:TPIECE])
            for d0, d1 in ((4, 12), (12, 20), (20, DC)):
                nc.sync.dma_start(wq_sb[:, d0:d1, :], wq_v[:, d0:d1, :])
            nc.sync.dma_start(xt00[:, 4:, :], xT_v[:, 4:, 0:TPIECE])
            rq_sb = wpool.tile([P, 2, T], BF16)
            nc.sync.dma_start(rq_sb[:], ropeq[:].rearrange("s p t -> p s t"))
            wk_sb = wpool.tile([P, DC, H], BF16)
            nc.sync.dma_start(wk_sb[:],
                              wk[:].rearrange("(c p) h -> p c h", p=P))
            wv_sb = wpool.tile([P, DC, H], BF16)
            nc.sync.dma_start(wv_sb[:],
                              wv[:].rearrange("(c p) h -> p c h", p=P))
            rk_sb = wpool.tile([P, 2, T], BF16)
            nc.sync.dma_start(rk_sb[:], ropek[:].rearrange("s p t -> p s t"))
            nc.sync.dma_start(msk_sb[:], msk[:].rearrange("m p q -> p m q"))

            for b in range(B):
                qt = qkvpool.tile([P, 4, T], BF16, tag=f"qt{b}")
                kt = qkvpool.tile([P, 2, T], BF16, tag=f"kt{b}")
                vsb = qkvpool.tile([P, TQ, H], BF16, tag=f"v{b}")
                qts.append(qt)
                kts.append(kt)
                vsbs.append(vsb)

            # =================== projections + rope =======================
            # qt chunk order is (c*2 + h): [c0h0, c0h1, c1h0, c1h1] so the
            # QK matmul rhs for contraction chunk c is a contiguous slice.
            def proj_piece(b, pi):
                ppsum, vpsum = P_POOLS["ppsum"], P_POOLS["vpsum"]
                qt, kt, vsb = qts[b], kts[b], vsbs[b]
                t0 = pi * TPIECE
                if b == 0 and pi == 0:
                    xt = xt00
                else:
                    xt = xpool.tile([P, DC, TPIECE], BF16, tag="xt")
                    nc.sync.dma_start(
                        xt[:], xT_v[:, :, b * T + t0:b * T + t0 + TPIECE])

                def rope(dst, i0, i1, psA, psB, tab):
                    cos = tab[:, 0, t0:t0 + TPIECE]
                    sin = tab[:, 1, t0:t0 + TPIECE]
                    t1 = rpool.tile([P, TPIECE], F32, tag="r1")
                    t2 = rpool.tile([P, TPIECE], F32, tag="r2")
                    nc.vector.tensor_tensor(t1[:], psA[:], cos, ALU.mult)
                    nc.vector.tensor_tensor(t2[:], psB[:], sin, ALU.mult)
                    nc.vector.tensor_tensor(
                        dst[:, i0, t0:t0 + TPIECE], t1[:], t2[:],
                        ALU.subtract)
                    t3 = rpool.tile([P, TPIECE], F32, tag="r3")
                    t4 = rpool.tile([P, TPIECE], F32, tag="r4")
                    nc.vector.tensor_tensor(t3[:], psB[:], cos, ALU.mult)
                    nc.vector.tensor_tensor(t4[:], psA[:], sin, ALU.mult)
                    nc.vector.tensor_tensor(
                        dst[:, i1, t0:t0 + TPIECE], t3[:], t4[:], ALU.add)

                # Q projections: head h -> chunks h (c0) and 2+h (c1)
                for hh in range(2):
                    ps = []
                    for cc in range(2):
                        hc = cc * 2 + hh
                        pq = ppsum.tile([P, TPIECE], F32, tag="pq")
                        for dc in range(DC):
                            nc.tensor.matmul(
                                pq[:],
                                wq_sb[:, dc, hc * P:(hc + 1) * P],
                                xt[:, dc, :],
                                start=(dc == 0), stop=(dc == DC - 1))
                        ps.append(pq)
                    rope(qt, hh, 2 + hh, ps[0], ps[1], rq_sb)
                # K projection: 2 h-chunks
                ps = []
                for half in range(2):
                    pk = ppsum.tile([P, TPIECE], F32, tag="pq")
                    for dc in range(DC):
                        nc.tensor.matmul(
                            pk[:],
                            wk_sb[:, dc, half * P:(half + 1) * P],
                            xt[:, dc, :],
                            start=(dc == 0), stop=(dc == DC - 1))
                    ps.append(pk)
                rope(kt, 0, 1, ps[0], ps[1], rk_sb)
                # V projection: natural layout [t, h]
                for tc4 in range(TPIECE // P):
                    pv = vpsum.tile([P, H], F32, tag="pv")
                    for dc in range(DC):
                        nc.tensor.matmul(
                            pv[:],
                            xt[:, dc, tc4 * P:(tc4 + 1) * P],
                            wv_sb[:, dc, :],
                            start=(dc == 0), stop=(dc == DC - 1))
                    nc.vector.tensor_copy(
                        out=vsb[:, pi * (TPIECE // P) + tc4, :], in_=pv[:])

            # ======================= attention ===========================
            # Stage 1 issues QK + the softmax activations; stage 2 (run
            # later, with other PE work in between to hide the Scalar-
            # engine latency) does denominators, PV, normalize and ship.
            def attend_stage1(b, i):
                qkps = P_POOLS["qkps"]
                qt, kt = qts[b], kts[b]
                lo, hi = _band(i, mode)
                nj = hi - lo + 1
                nseg = (nj + SEGJ - 1) // SEGJ
                es_slices = []
                for si in range(nseg):
                    j0 = lo + si * SEGJ
                    j1 = min(j0 + SEGJ - 1, hi)
                    w = (j1 - j0 + 1) * 2 * P
                    ts_ = apool.tile([P, SEGJ * 2 * P], F32, tag=f"ts{si}")
                    # QK for pairs of key tiles sharing one PSUM bank; a
                    # single Tanh drains each bank into the SBUF strip.
                    jlist = list(range(j0, j1 + 1))
                    for pj in range(0, len(jlist), 2):
                        jpair = jlist[pj:pj + 2]
                        qk = qkps.tile([P, 4 * P], F32, tag="qk")
                        for u, j in enumerate(jpair):
                            for c in range(2):
                                nc.tensor.matmul(
                                    qk[:, u * 2 * P:(u + 1) * 2 * P],
                                    kt[:, c, j * P:(j + 1) * P],
                                    qt[:, 2 * c:2 * c + 2,
                                       i * P:(i + 1) * P],
                                    start=(c == 0), stop=(c == 1))
                        w2 = len(jpair) * 2 * P
                        nc.scalar.activation(
                            ts_[:, pj * 2 * P:pj * 2 * P + w2],
                            qk[:, :w2], AF.Tanh, scale=1.0 / SOFT_CAP)
                    for jj, j in enumerate(jlist):
                        mi = None
                        if j == i - WTILES:
                            mi = 0
                        elif j == i and mode == "tril":
                            mi = 1
                        elif j == i + WTILES and mode == "ones":
                            mi = 2
                        if mi is not None:
                            sl = slice(jj * 2 * P, (jj + 1) * 2 * P)
                            nc.gpsimd.tensor_tensor(
                                ts_[:, sl], ts_[:, sl], msk_sb[:, mi, :],
                                ALU.add)
                    es = espool.tile([P, SEGJ * 2 * P], BF16, tag=f"es{si}")
                    nc.scalar.activation(es[:, :w], ts_[:, :w], AF.Exp,
                                         scale=SOFT_CAP)
                    for jj in range(len(jlist)):
                        es_slices.append(es[:, jj * 2 * P:(jj + 1) * 2 * P])
                return (b, i, lo, nj, es_slices)

            def attend_stage2(st):
                dnps, encps = P_POOLS["dnps"], P_POOLS["encps"]
                b, i, lo, nj, es_slices = st
                vsb = vsbs[b]
                # softmax denominators: dn[0, q] = sum_k es[k, q]
                dn = dnps.tile([1, 2 * P], F32, tag="dn")
                for jj, esl in enumerate(es_slices):
                    nc.tensor.matmul(dn[:], ones_col[:], esl,
                                     start=(jj == 0), stop=(jj == nj - 1))
                rinv = apool.tile([1, 2 * P], F32, tag="rinv")
                nc.vector.reciprocal_approx_fast(rinv[:], dn[:])
                rb = rbpool.tile([P, 2 * P], F32, tag="rb")
                nc.gpsimd.partition_broadcast(rb[:], rinv[:])
                # PV: encT chunks [hd_c, (h0|h1) q]; both c-chunks share one
                # PSUM bank. encsb is stored in a2a chunk order (h*2+c).
                enc = encps.tile([P, 4 * P], F32, tag="enc")
                encsb = apool.tile([P, 4, P], BF16, tag="encsb")
                encsb_v = encsb[:].rearrange("p (h c) t -> p c h t", h=2)
                for c in range(2):
                    for jj, esl in enumerate(es_slices):
                        j = lo + jj
                        nc.tensor.matmul(
                            enc[:, c * 2 * P:(c + 1) * 2 * P],
                            vsb[:, j, c * P:(c + 1) * P],
                            esl,
                            start=(jj == 0), stop=(jj == nj - 1))
                    nc.vector.tensor_tensor(
                        encsb_v[:, c],
                        enc[:, c * 2 * P:(c + 1) * 2 * P].rearrange(
                            "p (h t) -> p h t", h=2),
                        rb[:].rearrange("p (h t) -> p h t", h=2),
                        ALU.mult)
                gtok = b * T + i * P
                jb = gtok // TPC
                ch = (gtok % TPC) // CHTOK
                toff = gtok % CHTOK
                dst = a2a_in[ch][jb].rearrange(
                    "(c p) t -> p c t", p=P)[:, :, toff:toff + P]
                nc.sync.dma_start(dst, encsb[:])

            # ================ output projection helpers ===================
            wo_tiles = {}
            efs = {}
            O_POOLS = {}

            def load_wo(half, dp):
                t = O_POOLS["wo"].tile([P, NHC, DP], BF16, tag="wo")
                wv_ = wo[:, :, dp * DP:(dp + 1) * DP].rearrange(
                    "c p d -> p c d")
                for h0 in range(0, NHC, 8):
                    nc.gpsimd.dma_start(t[:, h0:h0 + 8, :],
                                        wv_[:, h0:h0 + 8, :])
                wo_tiles[(half, dp)] = t

            def load_ef(half):
                ef = O_POOLS["ef"].tile([P, NHC, CHTOK], BF16,
                                        tag=f"ef{half}")
                ev = a2a_out[half][:].rearrange("b (c p) t -> p (b c) t",
                                                p=P)
                for h0 in range(0, NHC, 8):
                    nc.gpsimd.dma_start(ef[:, h0:h0 + 8, :],
                                        ev[:, h0:h0 + 8, :])
                efs[half] = ef

            # ---- interleaved schedule ----
            # Every attend's stage 1 (QK + softmax issue) is emitted right
            # after the projection piece that completes its rows; stage 2
            # runs one piece later, when the softmax chain has certainly
            # drained. The b1 attends of A2A chunk 1 are held back to give
            # the PE work that covers A2A0 + the ef0/wo loads.
            ostack = contextlib.ExitStack()
            with (
                tc.tile_pool(name="proj_ps", bufs=3, space="PSUM") as ppsum,
                tc.tile_pool(name="projv_ps", bufs=1, space="PSUM") as vpsum,
                tc.tile_pool(name="qk_ps", bufs=2, space="PSUM") as qkps,
                tc.tile_pool(name="dn_ps", bufs=1, space="PSUM") as dnps,
                tc.tile_pool(name="enc_ps", bufs=1, space="PSUM") as encps,
            ):
                P_POOLS.update(ppsum=ppsum, vpsum=vpsum, qkps=qkps,
                               dnps=dnps, encps=encps)
                pend = []

                def flush():
                    while pend:
                        attend_stage2(pend.pop(0))

                for pi in range(NPIECE):
                    proj_piece(0, pi)
                    flush()
                    pend.append(attend_stage1(0, 2 * pi))
                    pend.append(attend_stage1(0, 2 * pi + 1))
                for pi in range(NPIECE):
                    proj_piece(1, pi)
                    flush()
                    if pi % 2 == 0:       # rows 2pi, 2pi+1 are chunk 0
                        pend.append(attend_stage1(1, 2 * pi))
                        pend.append(attend_stage1(1, 2 * pi + 1))
                flush()
                # projections done: free weights/x SBUF, open oproj pools
                pstack.close()
                O_POOLS["wo"] = ostack.enter_context(tc.tile_pool(
                    name="wo_sb", bufs=2 if mode == "tril" else 1))
                O_POOLS["ef"] = ostack.enter_context(
                    tc.tile_pool(name="ef", bufs=1))
                O_POOLS["osb"] = ostack.enter_context(
                    tc.tile_pool(name="osb", bufs=2))
                nc.gpsimd.collective_compute(
                    "AllToAll", ALU.bypass,
                    replica_groups=[list(range(NCORES))],
                    ins=[a2a_in[0][:].opt()], outs=[a2a_out[0][:].opt()])
                load_ef(0)
                load_wo(0, 0)
                load_wo(0, 1)
                for i in range(TQ):
                    if (i % 4) // 2 == 1:
                        while len(pend) >= 2:
                            attend_stage2(pend.pop(0))
                        pend.append(attend_stage1(1, i))
                flush()
                nc.gpsimd.collective_compute(
                    "AllToAll", ALU.bypass,
                    replica_groups=[list(range(NCORES))],
                    ins=[a2a_in[1][:].opt()], outs=[a2a_out[1][:].opt()])

            # ==================== output projection ====================
            with tc.tile_pool(name="oproj_ps", bufs=2,
                              space="PSUM") as opsum:
                for half in range(2):
                    if half == 1:
                        load_ef(1)
                    ef = efs[half]
                    for dp in range(D // DP):
                        if (half, dp) not in wo_tiles:
                            load_wo(half, dp)
                        wo_sb = wo_tiles[(half, dp)]
                        for t2 in range(2):
                            tc4 = half * 2 + t2
                            po = opsum.tile([P, DP], F32, tag="po")
                            for hc in range(NHC):
                                nc.tensor.matmul(
                                    po[:],
                                    ef[:, hc, t2 * P:(t2 + 1) * P],
                                    wo_sb[:, hc, :],
                                    start=(hc == 0), stop=(hc == NHC - 1))
                            osb = O_POOLS["osb"].tile([P, DP], F32,
                                                      tag="osb")
                            nc.scalar.activation(osb[:], po[:], AF.Copy)
                            nc.sync.dma_start(
                                out[tc4 * P:(tc4 + 1) * P,
                                    dp * DP:(dp + 1) * DP],
                                osb[:])
            ostack.close()

    nc.compile()
    return nc


def _rope_tables(pos, scale):
    """pos: [T] int array -> [2, 128, T] bf16 (cos;sin), scaled."""
    frac = 2.0 * np.arange(H // 2, dtype=np.float64) / H
    timescale = ROPE_BASE ** frac                      # [128]
    sinusoid = pos.astype(np.float64)[None, :] / timescale[:, None]  # [128,T]
    tabs = np.stack([np.cos(sinusoid), np.sin(sinusoid)]) * scale
    return tabs.astype(ml_dtypes.bfloat16)


def _masks():
    """[3, 128, 256] f32 additive masks in [k, q(2 heads)] layout."""
    kl = np.arange(P)[:, None]
    ql = np.arange(2 * P)[None, :] % P
    m0 = np.where(kl > ql, 0.0, MASKVAL)    # j == i-8: valid k_l > q_l
    m1 = np.where(kl <= ql, 0.0, MASKVAL)   # j == i (causal): valid k_l <= q_l
    m2 = np.where(kl < ql, 0.0, MASKVAL)    # j == i+8: valid k_l < q_l
    return np.stack([m0, m1, m2]).astype(np.float32)


def _reference_host(x, segment_pos, attn_mask, w_q, w_kv, w_o):
    """Slow but fully general fallback (numpy)."""
    xb = x.astype(np.float32)
    q = np.einsum('btd,ndh->btnh', xb, w_q)
    k = np.einsum('bsd,kdh->bskh', xb, w_kv[0])
    v = np.einsum('bsd,kdh->bskh', xb, w_kv[1])

    def rope(t, positions):
        hd = t.shape[-1]
        frac = 2.0 * np.arange(hd // 2, dtype=np.float32) / hd
        ts_ = ROPE_BASE ** frac
        sinusoid = positions.astype(np.float32)[..., None] / ts_
        sinusoid = sinusoid[..., None, :]
        s, c = np.sin(sinusoid), np.cos(sinusoid)
        first, second = np.split(t, 2, axis=-1)
        return np.concatenate([first * c - second * s,
                               second * c + first * s], axis=-1)

    q = rope(q, segment_pos) * SCALAR
    k = rope(k, segment_pos)
    qg = q.reshape(B, T, NKV, 2, H)
    logits = np.einsum('btkgh,bskh->btkgs', qg, k).reshape(B, T, NQ, T)
    logits = np.tanh(logits / SOFT_CAP) * SOFT_CAP
    pos_s = np.arange(T)[None, None, :]
    pos_t = segment_pos[:, :, None]
    sliding = (pos_s > pos_t - WINDOW) & (pos_s < pos_t + WINDOW)
    mask = np.logical_and(attn_mask, sliding)
    padded = np.where(mask[:, :, None, :], logits, -np.inf)
    padded -= padded.max(axis=-1, keepdims=True)
    e = np.exp(padded)
    probs = (e / e.sum(axis=-1, keepdims=True)).astype(np.float32)
    v_exp = np.repeat(v, NQ // NKV, axis=2)            # [B,T,NQ,H]
    enc = np.einsum('btns,bsnh->btnh', probs, v_exp)
    return np.einsum('btnh,nhd->btd', enc, w_o).astype(np.float32)


_GRAPH_CACHE = {}


def kernel(x, segment_pos, attn_mask, w_q, w_kv, w_o):
    global last_result
    x = np.asarray(x)
    segment_pos = np.asarray(segment_pos)
    attn_mask = np.asarray(attn_mask)
    w_q = np.asarray(w_q, dtype=np.float32)
    w_kv = np.asarray(w_kv, dtype=np.float32)
    w_o = np.asarray(w_o, dtype=np.float32)

    arange = np.broadcast_to(np.arange(T, dtype=segment_pos.dtype), (B, T))
    std_pos = np.array_equal(segment_pos, arange)
    tril = np.broadcast_to(np.tril(np.ones((T, T), dtype=bool)), (B, T, T))
    if attn_mask.all():
        mode = "ones"
    elif np.array_equal(attn_mask, tril):
        mode = "tril"
    else:
        mode = None
    if not std_pos or mode is None:
        return _reference_host(x, segment_pos, attn_mask, w_q, w_kv, w_o)

    if mode not in _GRAPH_CACHE:
        _GRAPH_CACHE[mode] = build(mode)
    nc = _GRAPH_CACHE[mode]

    bf = ml_dtypes.bfloat16
    xT = np.ascontiguousarray(x.reshape(TOK, D).T).astype(bf)    # [D, TOK]
    pos = segment_pos[0]
    ropeq = np.ascontiguousarray(_rope_tables(pos, SCALAR))
    ropek = np.ascontiguousarray(_rope_tables(pos, 1.0))
    wo_all = np.ascontiguousarray(
        w_o.reshape(NHC, P, D)).astype(bf)
    msk = np.ascontiguousarray(_masks())

    in_maps = []
    for c in range(NCORES):
        # wq columns in qt chunk order (c*2 + h): [c0h0, c0h1, c1h0, c1h1]
        g0, g1 = w_q[2 * c], w_q[2 * c + 1]
        wq_c = np.ascontiguousarray(
            np.concatenate([g0[:, :P], g1[:, :P], g0[:, P:], g1[:, P:]],
                           axis=1)).astype(bf)
        wk_c = np.ascontiguousarray(w_kv[0, c]).astype(bf)
        wv_c = np.ascontiguousarray(w_kv[1, c]).astype(bf)
        in_maps.append({
            "xT": xT, "wq": wq_c, "wk": wk_c, "wv": wv_c, "wo": wo_all,
            "ropeq": ropeq, "ropek": ropek, "msk": msk,
        })

    trace = os.environ.get("KTRACE", "0") == "1"
    res = run_bass_kernel_spmd(nc, in_maps, core_ids=list(range(NCORES)),
                               trace=trace)
    last_result = res
    outs = [res.results[c]["out"] for c in range(NCORES)]
    return np.concatenate(outs, axis=0).reshape(B, T, D).astype(np.float32)


# revision 31
# speedup vs baseline: 1.2012x; 1.2012x over previous
"""Sliding-window GQA attention (Gemma-style) on 8 TRN2 NeuronCores.

Sharding: tensor-parallel over heads. Core c owns q-heads {2c, 2c+1} and
kv-head c. Each core computes Q/K/V projections (+RoPE) for its heads over
the full sequence, banded sliding-window attention, then an AllToAll
(split into 2 token-chunks) reshards the attention output by token so
every core computes the full output projection for its 512-token slice.
Host concatenates slices.

Attention uses a [k, q] logits layout (K stationary, Q moving) so the
softmax probabilities leave the Scalar engine directly in the [k, q]
orientation PV needs -- no PE transposes. QK logits for two adjacent key
tiles share one PSUM bank so one Tanh activation drains them straight
into a wide SBUF strip; one Exp activation per band segment produces
bf16 probabilities. Window/causal masks are added by GpSimd. Softmax
denominators come from ones-vector matmuls accumulated in PSUM and the
normalization is applied once on the (much smaller) attention output.

Batch-0 attention is interleaved into batch-1's projection so the
Scalar/Vector-engine softmax work hides under projection matmuls.

All matmuls run in bf16 (f32 PSUM accumulation); softmax runs in f32.
"""

import contextlib
import os
import sys

for _p in ("/opt/trn_rl_repo",):
    if _p not in sys.path:
        sys.path.insert(0, _p)

import numpy as np
import ml_dtypes

import concourse.bass as bass
import concourse.mybir as mybir
import concourse.tile as tile
from concourse import bacc
from concourse.bass_utils import run_bass_kernel_spmd

F32 = mybir.dt.float32
BF16 = mybir.dt.bfloat16
AF = mybir.ActivationFunctionType
ALU = mybir.AluOpType

B, T, D = 2, 2048, 3584
NQ, NKV, H = 16, 8, 256
SCALAR = 0.0625
SOFT_CAP = 50.0
WINDOW = 1024
ROPE_BASE = 10000.0

NCORES = 8
P = 128
DC = D // P              # 28 contraction chunks
TQ = T // P              # 16 query tiles per batch
TPIECE = 256             # projection output tile width
NPIECE = T // TPIECE
HLOC = 2 * H             # 512 local q-head columns per core
TOK = B * T              # 4096
TPC = TOK // NCORES      # 512 tokens per core after AllToAll
CHTOK = TPC // 2         # 256 tokens per A2A chunk block
WTILES = WINDOW // P     # 8
MASKVAL = -1.0e30        # added to tanh output; exp(50*(t+MASKVAL)) == 0
NHC = NQ * H // P        # 32 global h chunks
DP = 512                 # output projection d piece
SEGJ = 9                 # max band tiles per softmax strip

last_result = None       # BassKernelResults of the most recent device run


def _band(i, mode):
    lo = max(0, i - WTILES)
    hi = i if mode == "tril" else min(TQ - 1, i + WTILES)
    return lo, hi


def build(mode):
    assert mode in ("tril", "ones")
    nc = bacc.Bacc("TRN2", target_bir_lowering=False, debug=False,
                   num_devices=NCORES)

    xT = nc.dram_tensor("xT", [D, TOK], BF16, kind="ExternalInput")
    wq = nc.dram_tensor("wq", [D, HLOC], BF16, kind="ExternalInput")
    wk = nc.dram_tensor("wk", [D, H], BF16, kind="ExternalInput")
    wv = nc.dram_tensor("wv", [D, H], BF16, kind="ExternalInput")
    wo = nc.dram_tensor("wo", [NHC, P, D], BF16, kind="ExternalInput")
    ropeq = nc.dram_tensor("ropeq", [2, P, T], BF16, kind="ExternalInput")
    ropek = nc.dram_tensor("ropek", [2, P, T], BF16, kind="ExternalInput")
    msk = nc.dram_tensor("msk", [3, P, 2 * P], F32, kind="ExternalInput")
    out = nc.dram_tensor("out", [TPC, D], F32, kind="ExternalOutput")

    with tile.TileContext(nc) as tc:
        with (
            tc.tile_pool(name="dram", bufs=1, space="DRAM") as dram,
            tc.tile_pool(name="consts", bufs=1) as consts,
            tc.tile_pool(name="qkv", bufs=1) as qkvpool,
            tc.tile_pool(name="attn_sb", bufs=2) as apool,
            tc.tile_pool(name="es_sb", bufs=3) as espool,
            tc.tile_pool(name="rb_sb", bufs=1) as rbpool,
        ):
            # A2A bounce buffers: [src_rank_block][local h chunk-major][tok]
            a2a_in = [dram.tile([NCORES, HLOC, CHTOK], BF16,
                                name=f"a2a_in{m}") for m in range(2)]
            a2a_out = [dram.tile([NCORES, HLOC, CHTOK], BF16,
                                 name=f"a2a_out{m}") for m in range(2)]

            ones_col = consts.tile([P, 1], BF16)
            nc.gpsimd.memset(ones_col, 1.0)
            msk_sb = consts.tile([P, 3, 2 * P], F32)

            qts, kts, vsbs = [], [], []
            P_POOLS = {}
            xT_v = xT[:].rearrange("(c p) t -> p c t", p=P)

            # weights/x pools close mid-schedule (after projections) so
            # their SBUF space is reused by the wo/ef pools.
            pstack = contextlib.ExitStack()
            wpool = pstack.enter_context(
                tc.tile_pool(name="weights", bufs=1))
            xpool = pstack.enter_context(tc.tile_pool(name="xt", bufs=2))
            rpool = pstack.enter_context(tc.tile_pool(name="rtmp", bufs=1))

            # ---- staged input loads: first slices small so the first
            # matmuls start within a few us of kernel start ----
            wq_sb = wpool.tile([P, DC, HLOC], BF16)
            wq_v = wq[:].rearrange("(c p) h -> p c h", p=P)
            nc.sync.dma_start(wq_sb[:, 0:4, :], wq_v[:, 0:4, :])
            xt00 = xpool.tile([P, DC, TPIECE], BF16, tag="xt")
            nc.sync.dma_start(xt00[:, 0:4, :], xT_v[:, 0:4, 0:TPIECE])
            for d0, d1 in ((4, 12), (12, 20), (20, DC)):
                nc.sync.dma_start(wq_sb[:, d0:d1, :], wq_v[:, d0:d1, :])
            nc.sync.dma_start(xt00[:, 4:, :], xT_v[:, 4:, 0:TPIECE])
            rq_sb = wpool.tile([P, 2, T], BF16)
            nc.sync.dma_start(rq_sb[:], ropeq[:].rearrange("s p t -> p s t"))
            wk_sb = wpool.tile([P, DC, H], BF16)
            nc.sync.dma_start(wk_sb[:],
                              wk[:].rearrange("(c p) h -> p c h", p=P))
            wv_sb = wpool.tile([P, DC, H], BF16)
            nc.sync.dma_start(wv_sb[:],
                              wv[:].rearrange("(c p) h -> p c h", p=P))
            rk_sb = wpool.tile([P, 2, T], BF16)
            nc.sync.dma_start(rk_sb[:], ropek[:].rearrange("s p t -> p s t"))
            nc.sync.dma_start(msk_sb[:], msk[:].rearrange("m p q -> p m q"))

            for b in range(B):
                qt = qkvpool.tile([P, 4, T], BF16, tag=f"qt{b}")
                kt = qkvpool.tile([P, 2, T], BF16, tag=f"kt{b}")
                vsb = qkvpool.tile([P, TQ, H], BF16, tag=f"v{b}")
                qts.append(qt)
                kts.append(kt)
                vsbs.append(vsb)

            # =================== projections + rope =======================
            # qt chunk order is (c*2 + h): [c0h0, c0h1, c1h0, c1h1] so the
            # QK matmul rhs for contraction chunk c is a contiguous slice.
            def proj_piece(b, pi):
                ppsum, vpsum = P_POOLS["ppsum"], P_POOLS["vpsum"]
                qt, kt, vsb = qts[b], kts[b], vsbs[b]
                t0 = pi * TPIECE
                if b == 0 and pi == 0:
                    xt = xt00
                else:
                    xt = xpool.tile([P, DC, TPIECE], BF16, tag="xt")
                    nc.sync.dma_start(
                        xt[:], xT_v[:, :, b * T + t0:b * T + t0 + TPIECE])

                def rope(dst, i0, i1, psA, psB, tab):
                    cos = tab[:, 0, t0:t0 + TPIECE]
                    sin = tab[:, 1, t0:t0 + TPIECE]
                    t1 = rpool.tile([P, TPIECE], F32, tag="r1")
                    t2 = rpool.tile([P, TPIECE], F32, tag="r2")
                    nc.vector.tensor_tensor(t1[:], psA[:], cos, ALU.mult)
                    nc.vector.tensor_tensor(t2[:], psB[:], sin, ALU.mult)
                    nc.vector.tensor_tensor(
                        dst[:, i0, t0:t0 + TPIECE], t1[:], t2[:],
                        ALU.subtract)
                    t3 = rpool.tile([P, TPIECE], F32, tag="r3")
                    t4 = rpool.tile([P, TPIECE], F32, tag="r4")
                    nc.vector.tensor_tensor(t3[:], psB[:], cos, ALU.mult)
                    nc.vector.tensor_tensor(t4[:], psA[:], sin, ALU.mult)
                    nc.vector.tensor_tensor(
                        dst[:, i1, t0:t0 + TPIECE], t3[:], t4[:], ALU.add)

                # Q projections: head h -> chunks h (c0) and 2+h (c1)
                for hh in range(2):
                    ps = []
                    for cc in range(2):
                        hc = cc * 2 + hh
                        pq = ppsum.tile([P, TPIECE], F32, tag="pq")
                        for dc in range(DC):
                            nc.tensor.matmul(
                                pq[:],
                                wq_sb[:, dc, hc * P:(hc + 1) * P],
                                xt[:, dc, :],
                                start=(dc == 0), stop=(dc == DC - 1))
                        ps.append(pq)
                    rope(qt, hh, 2 + hh, ps[0], ps[1], rq_sb)
                # K projection: 2 h-chunks
                ps = []
                for half in range(2):
                    pk = ppsum.tile([P, TPIECE], F32, tag="pq")
                    for dc in range(DC):
                        nc.tensor.matmul(
                            pk[:],
                            wk_sb[:, dc, half * P:(half + 1) * P],
                            xt[:, dc, :],
                            start=(dc == 0), stop=(dc == DC - 1))
                    ps.append(pk)
                rope(kt, 0, 1, ps[0], ps[1], rk_sb)
                # V projection: natural layout [t, h]
                for tc4 in range(TPIECE // P):
                    pv = vpsum.tile([P, H], F32, tag="pv")
                    for dc in range(DC):
                        nc.tensor.matmul(
                            pv[:],
                            xt[:, dc, tc4 * P:(tc4 + 1) * P],
                            wv_sb[:, dc, :],
                            start=(dc == 0), stop=(dc == DC - 1))
                    nc.vector.tensor_copy(
                        out=vsb[:, pi * (TPIECE // P) + tc4, :], in_=pv[:])

            # ======================= attention ===========================
            # Stage 1 issues QK + the softmax activations; stage 2 (run
            # later, with other PE work in between to hide the Scalar-
            # engine latency) does denominators, PV, normalize and ship.
            def attend_stage1(b, i):
                qkps = P_POOLS["qkps"]
                qt, kt = qts[b], kts[b]
                lo, hi = _band(i, mode)
                nj = hi - lo + 1
                nseg = (nj + SEGJ - 1) // SEGJ
                es_slices = []
                for si in range(nseg):
                    j0 = lo + si * SEGJ
                    j1 = min(j0 + SEGJ - 1, hi)
                    w = (j1 - j0 + 1) * 2 * P
                    ts_ = apool.tile([P, SEGJ * 2 * P], F32, tag=f"ts{si}")
                    # QK for pairs of key tiles sharing one PSUM bank; a
                    # single Tanh drains each bank into the SBUF strip.
                    jlist = list(range(j0, j1 + 1))
                    for pj in range(0, len(jlist), 2):
                        jpair = jlist[pj:pj + 2]
                        qk = qkps.tile([P, 4 * P], F32, tag="qk")
                        for u, j in enumerate(jpair):
                            for c in range(2):
                                nc.tensor.matmul(
                                    qk[:, u * 2 * P:(u + 1) * 2 * P],
                                    kt[:, c, j * P:(j + 1) * P],
                                    qt[:, 2 * c:2 * c + 2,
                                       i * P:(i + 1) * P],
                                    start=(c == 0), stop=(c == 1))
                        w2 = len(jpair) * 2 * P
                        nc.scalar.activation(
                            ts_[:, pj * 2 * P:pj * 2 * P + w2],
                            qk[:, :w2], AF.Tanh, scale=1.0 / SOFT_CAP)
                    for jj, j in enumerate(jlist):
                        mi = None
                        if j == i - WTILES:
                            mi = 0
                        elif j == i and mode == "tril":
                            mi = 1
                        elif j == i + WTILES and mode == "ones":
                            mi = 2
                        if mi is not None:
                            sl = slice(jj * 2 * P, (jj + 1) * 2 * P)
                            nc.vector.tensor_tensor(
                                ts_[:, sl], ts_[:, sl], msk_sb[:, mi, :],
                                ALU.add)
                    es = espool.tile([P, SEGJ * 2 * P], BF16, tag=f"es{si}")
                    nc.scalar.activation(es[:, :w], ts_[:, :w], AF.Exp,
                                         scale=SOFT_CAP)
                    for jj in range(len(jlist)):
                        es_slices.append(es[:, jj * 2 * P:(jj + 1) * 2 * P])
                return (b, i, lo, nj, es_slices)

            def attend_stage2(st):
                dnps, encps = P_POOLS["dnps"], P_POOLS["encps"]
                b, i, lo, nj, es_slices = st
                vsb = vsbs[b]
                # softmax denominators: dn[0, q] = sum_k es[k, q]
                dn = dnps.tile([1, 2 * P], F32, tag="dn")
                for jj, esl in enumerate(es_slices):
                    nc.tensor.matmul(dn[:], ones_col[:], esl,
                                     start=(jj == 0), stop=(jj == nj - 1))
                rinv = apool.tile([1, 2 * P], F32, tag="rinv")
                nc.vector.reciprocal_approx_fast(rinv[:], dn[:])
                rb = rbpool.tile([P, 2 * P], F32, tag="rb")
                nc.gpsimd.partition_broadcast(rb[:], rinv[:])
                # PV: encT chunks [hd_c, (h0|h1) q]; both c-chunks share one
                # PSUM bank. encsb is stored in a2a chunk order (h*2+c).
                enc = encps.tile([P, 4 * P], F32, tag="enc")
                encsb = apool.tile([P, 4, P], BF16, tag="encsb")
                encsb_v = encsb[:].rearrange("p (h c) t -> p c h t", h=2)
                for c in range(2):
                    for jj, esl in enumerate(es_slices):
                        j = lo + jj
                        nc.tensor.matmul(
                            enc[:, c * 2 * P:(c + 1) * 2 * P],
                            vsb[:, j, c * P:(c + 1) * P],
                            esl,
                            start=(jj == 0), stop=(jj == nj - 1))
                    nc.vector.tensor_tensor(
                        encsb_v[:, c],
                        enc[:, c * 2 * P:(c + 1) * 2 * P].rearrange(
                            "p (h t) -> p h t", h=2),
                        rb[:].rearrange("p (h t) -> p h t", h=2),
                        ALU.mult)
                gtok = b * T + i * P
                jb = gtok // TPC
                ch = (gtok % TPC) // CHTOK
                toff = gtok % CHTOK
                dst = a2a_in[ch][jb].rearrange(
                    "(c p) t -> p c t", p=P)[:, :, toff:toff + P]
                nc.sync.dma_start(dst, encsb[:])

            # ================ output projection helpers ===================
            wo_tiles = {}
            efs = {}
            O_POOLS = {}

            def load_wo(half, dp):
                t = O_POOLS["wo"].tile([P, NHC, DP], BF16, tag="wo")
                wv_ = wo[:, :, dp * DP:(dp + 1) * DP].rearrange(
                    "c p d -> p c d")
                for h0 in range(0, NHC, 8):
                    nc.gpsimd.dma_start(t[:, h0:h0 + 8, :],
                                        wv_[:, h0:h0 + 8, :])
                wo_tiles[(half, dp)] = t

            def load_ef(half):
                ef = O_POOLS["ef"].tile([P, NHC, CHTOK], BF16,
                                        tag=f"ef{half}")
                ev = a2a_out[half][:].rearrange("b (c p) t -> p (b c) t",
                                                p=P)
                for h0 in range(0, NHC, 8):
                    nc.gpsimd.dma_start(ef[:, h0:h0 + 8, :],
                                        ev[:, h0:h0 + 8, :])
                efs[half] = ef

            # ---- interleaved schedule ----
            # Every attend's stage 1 (QK + softmax issue) is emitted right
            # after the projection piece that completes its rows; stage 2
            # runs one piece later, when the softmax chain has certainly
            # drained. The b1 attends of A2A chunk 1 are held back to give
            # the PE work that covers A2A0 + the ef0/wo loads.
            ostack = contextlib.ExitStack()
            with (
                tc.tile_pool(name="proj_ps", bufs=3, space="PSUM") as ppsum,
                tc.tile_pool(name="projv_ps", bufs=1, space="PSUM") as vpsum,
                tc.tile_pool(name="qk_ps", bufs=2, space="PSUM") as qkps,
                tc.tile_pool(name="dn_ps", bufs=1, space="PSUM") as dnps,
                tc.tile_pool(name="enc_ps", bufs=1, space="PSUM") as encps,
            ):
                P_POOLS.update(ppsum=ppsum, vpsum=vpsum, qkps=qkps,
                               dnps=dnps, encps=encps)
                pend = []

                def flush():
                    while pend:
                        attend_stage2(pend.pop(0))

                for pi in range(NPIECE):
                    proj_piece(0, pi)
                    flush()
                    pend.append(attend_stage1(0, 2 * pi))
                    pend.append(attend_stage1(0, 2 * pi + 1))
                for pi in range(NPIECE):
                    proj_piece(1, pi)
                    flush()
                    if pi % 2 == 0:       # rows 2pi, 2pi+1 are chunk 0
                        pend.append(attend_stage1(1, 2 * pi))
                        pend.append(attend_stage1(1, 2 * pi + 1))
                flush()
                # projections done: free weights/x SBUF, open oproj pools
                pstack.close()
                O_POOLS["wo"] = ostack.enter_context(tc.tile_pool(
                    name="wo_sb", bufs=2 if mode == "tril" else 1))
                O_POOLS["ef"] = ostack.enter_context(
                    tc.tile_pool(name="ef", bufs=1))
                O_POOLS["osb"] = ostack.enter_context(
                    tc.tile_pool(name="osb", bufs=2))
                nc.gpsimd.collective_compute(
                    "AllToAll", ALU.bypass,
                    replica_groups=[list(range(NCORES))],
                    ins=[a2a_in[0][:].opt()], outs=[a2a_out[0][:].opt()])
                load_ef(0)
                load_wo(0, 0)
                load_wo(0, 1)
                for i in range(TQ):
                    if (i % 4) // 2 == 1:
                        while len(pend) >= 2:
                            attend_stage2(pend.pop(0))
                        pend.append(attend_stage1(1, i))
                flush()
                nc.gpsimd.collective_compute(
                    "AllToAll", ALU.bypass,
                    replica_groups=[list(range(NCORES))],
                    ins=[a2a_in[1][:].opt()], outs=[a2a_out[1][:].opt()])

            # ==================== output projection ====================
            with tc.tile_pool(name="oproj_ps", bufs=2,
                              space="PSUM") as opsum:
                for half in range(2):
                    if half == 1:
                        load_ef(1)
                    ef = efs[half]
                    for dp in range(D // DP):
                        if (half, dp) not in wo_tiles:
                            load_wo(half, dp)
                        wo_sb = wo_tiles[(half, dp)]
                        for t2 in range(2):
                            tc4 = half * 2 + t2
                            po = opsum.tile([P, DP], F32, tag="po")
                            for hc in range(NHC):
                                nc.tensor.matmul(
                                    po[:],
                                    ef[:, hc, t2 * P:(t2 + 1) * P],
                                    wo_sb[:, hc, :],
                                    start=(hc == 0), stop=(hc == NHC - 1))
                            osb = O_POOLS["osb"].tile([P, DP], F32,
                                                      tag="osb")
                            nc.scalar.activation(osb[:], po[:], AF.Copy)
                            nc.sync.dma_start(
                                out[tc4 * P:(tc4 + 1) * P,
                                    dp * DP:(dp + 1) * DP],
                                osb[:])
            ostack.close()

    nc.compile()
    return nc


def _rope_tables(pos, scale):
    """pos: [T] int array -> [2, 128, T] bf16 (cos;sin), scaled."""
    frac = 2.0 * np.arange(H // 2, dtype=np.float64) / H
    timescale = ROPE_BASE ** frac                      # [128]
    sinusoid = pos.astype(np.float64)[None, :] / timescale[:, None]  # [128,T]
    tabs = np.stack([np.cos(sinusoid), np.sin(sinusoid)]) * scale
    return tabs.astype(ml_dtypes.bfloat16)


def _masks():
    """[3, 128, 256] f32 additive masks in [k, q(2 heads)] layout."""
    kl = np.arange(P)[:, None]
    ql = np.arange(2 * P)[None, :] % P
    m0 = np.where(kl > ql, 0.0, MASKVAL)    # j == i-8: valid k_l > q_l
    m1 = np.where(kl <= ql, 0.0, MASKVAL)   # j == i (causal): valid k_l <= q_l
    m2 = np.where(kl < ql, 0.0, MASKVAL)    # j == i+8: valid k_l < q_l
    return np.stack([m0, m1, m2]).astype(np.float32)


def _reference_host(x, segment_pos, attn_mask, w_q, w_kv, w_o):
    """Slow but fully general fallback (numpy)."""
    xb = x.astype(np.float32)
    q = np.einsum('btd,ndh->btnh', xb, w_q)
    k = np.einsum('bsd,kdh->bskh', xb, w_kv[0])
    v = np.einsum('bsd,kdh->bskh', xb, w_kv[1])

    def rope(t, positions):
        hd = t.shape[-1]
        frac = 2.0 * np.arange(hd // 2, dtype=np.float32) / hd
        ts_ = ROPE_BASE ** frac
        sinusoid = positions.astype(np.float32)[..., None] / ts_
        sinusoid = sinusoid[..., None, :]
        s, c = np.sin(sinusoid), np.cos(sinusoid)
        first, second = np.split(t, 2, axis=-1)
        return np.concatenate([first * c - second * s,
                               second * c + first * s], axis=-1)

    q = rope(q, segment_pos) * SCALAR
    k = rope(k, segment_pos)
    qg = q.reshape(B, T, NKV, 2, H)
    logits = np.einsum('btkgh,bskh->btkgs', qg, k).reshape(B, T, NQ, T)
    logits = np.tanh(logits / SOFT_CAP) * SOFT_CAP
    pos_s = np.arange(T)[None, None, :]
    pos_t = segment_pos[:, :, None]
    sliding = (pos_s > pos_t - WINDOW) & (pos_s < pos_t + WINDOW)
    mask = np.logical_and(attn_mask, sliding)
    padded = np.where(mask[:, :, None, :], logits, -np.inf)
    padded -= padded.max(axis=-1, keepdims=True)
    e = np.exp(padded)
    probs = (e / e.sum(axis=-1, keepdims=True)).astype(np.float32)
    v_exp = np.repeat(v, NQ // NKV, axis=2)            # [B,T,NQ,H]
    enc = np.einsum('btns,bsnh->btnh', probs, v_exp)
    return np.einsum('btnh,nhd->btd', enc, w_o).astype(np.float32)


_GRAPH_CACHE = {}


def kernel(x, segment_pos, attn_mask, w_q, w_kv, w_o):
    global last_result
    x = np.asarray(x)
    segment_pos = np.asarray(segment_pos)
    attn_mask = np.asarray(attn_mask)
    w_q = np.asarray(w_q, dtype=np.float32)
    w_kv = np.asarray(w_kv, dtype=np.float32)
    w_o = np.asarray(w_o, dtype=np.float32)

    arange = np.broadcast_to(np.arange(T, dtype=segment_pos.dtype), (B, T))
    std_pos = np.array_equal(segment_pos, arange)
    tril = np.broadcast_to(np.tril(np.ones((T, T), dtype=bool)), (B, T, T))
    if attn_mask.all():
        mode = "ones"
    elif np.array_equal(attn_mask, tril):
        mode = "tril"
    else:
        mode = None
    if not std_pos or mode is None:
        return _reference_host(x, segment_pos, attn_mask, w_q, w_kv, w_o)

    if mode not in _GRAPH_CACHE:
        _GRAPH_CACHE[mode] = build(mode)
    nc = _GRAPH_CACHE[mode]

    bf = ml_dtypes.bfloat16
    xT = np.ascontiguousarray(x.reshape(TOK, D).T).astype(bf)    # [D, TOK]
    pos = segment_pos[0]
    ropeq = np.ascontiguousarray(_rope_tables(pos, SCALAR))
    ropek = np.ascontiguousarray(_rope_tables(pos, 1.0))
    wo_all = np.ascontiguousarray(
        w_o.reshape(NHC, P, D)).astype(bf)
    msk = np.ascontiguousarray(_masks())

    in_maps = []
    for c in range(NCORES):
        # wq columns in qt chunk order (c*2 + h): [c0h0, c0h1, c1h0, c1h1]
        g0, g1 = w_q[2 * c], w_q[2 * c + 1]
        wq_c = np.ascontiguousarray(
            np.concatenate([g0[:, :P], g1[:, :P], g0[:, P:], g1[:, P:]],
                           axis=1)).astype(bf)
        wk_c = np.ascontiguousarray(w_kv[0, c]).astype(bf)
        wv_c = np.ascontiguousarray(w_kv[1, c]).astype(bf)
        in_maps.append({
            "xT": xT, "wq": wq_c, "wk": wk_c, "wv": wv_c, "wo": wo_all,
            "ropeq": ropeq, "ropek": ropek, "msk": msk,
        })

    trace = os.environ.get("KTRACE", "0") == "1"
    res = run_bass_kernel_spmd(nc, in_maps, core_ids=list(range(NCORES)),
                               trace=trace)
    last_result = res
    outs = [res.results[c]["out"] for c in range(NCORES)]
    return np.concatenate(outs, axis=0).reshape(B, T, D).astype(np.float32)


# revision 32
# speedup vs baseline: 1.2204x; 1.0160x over previous
"""Sliding-window GQA attention (Gemma-style) on 8 TRN2 NeuronCores.

Sharding: tensor-parallel over heads. Core c owns q-heads {2c, 2c+1} and
kv-head c. Each core computes Q/K/V projections (+RoPE) for its heads over
the full sequence, banded sliding-window attention, then an AllToAll
(split into 2 token-chunks) reshards the attention output by token so
every core computes the full output projection for its 512-token slice.
Host concatenates slices.

Attention uses a [k, q] logits layout (K stationary, Q moving) so the
softmax probabilities leave the Scalar engine directly in the [k, q]
orientation PV needs -- no PE transposes. QK logits for two adjacent key
tiles share one PSUM bank so one Tanh activation drains them straight
into a wide SBUF strip; one Exp activation per band segment produces
bf16 probabilities. Window/causal masks are added by GpSimd. Softmax
denominators come from ones-vector matmuls accumulated in PSUM and the
normalization is applied once on the (much smaller) attention output.

Batch-0 attention is interleaved into batch-1's projection so the
Scalar/Vector-engine softmax work hides under projection matmuls.

All matmuls run in bf16 (f32 PSUM accumulation); softmax runs in f32.
"""

import contextlib
import os
import sys

for _p in ("/opt/trn_rl_repo",):
    if _p not in sys.path:
        sys.path.insert(0, _p)

import numpy as np
import ml_dtypes

import concourse.bass as bass
import concourse.mybir as mybir
import concourse.tile as tile
from concourse import bacc
from concourse.bass_utils import run_bass_kernel_spmd

F32 = mybir.dt.float32
BF16 = mybir.dt.bfloat16
AF = mybir.ActivationFunctionType
ALU = mybir.AluOpType

B, T, D = 2, 2048, 3584
NQ, NKV, H = 16, 8, 256
SCALAR = 0.0625
SOFT_CAP = 50.0
WINDOW = 1024
ROPE_BASE = 10000.0

NCORES = 8
P = 128
DC = D // P              # 28 contraction chunks
TQ = T // P              # 16 query tiles per batch
TPIECE = 256             # projection output tile width
NPIECE = T // TPIECE
HLOC = 2 * H             # 512 local q-head columns per core
TOK = B * T              # 4096
TPC = TOK // NCORES      # 512 tokens per core after AllToAll
CHTOK = TPC // 2         # 256 tokens per A2A chunk block
WTILES = WINDOW // P     # 8
MASKVAL = -1.0e30        # added to tanh output; exp(50*(t+MASKVAL)) == 0
NHC = NQ * H // P        # 32 global h chunks
DP = 512                 # output projection d piece
SEGJ = 9                 # max band tiles per softmax strip

last_result = None       # BassKernelResults of the most recent device run


def _band(i, mode):
    lo = max(0, i - WTILES)
    hi = i if mode == "tril" else min(TQ - 1, i + WTILES)
    return lo, hi


def build(mode):
    assert mode in ("tril", "ones")
    nc = bacc.Bacc("TRN2", target_bir_lowering=False, debug=False,
                   num_devices=NCORES)

    xT = nc.dram_tensor("xT", [D, TOK], BF16, kind="ExternalInput")
    wq = nc.dram_tensor("wq", [D, HLOC], BF16, kind="ExternalInput")
    wk = nc.dram_tensor("wk", [D, H], BF16, kind="ExternalInput")
    wv = nc.dram_tensor("wv", [D, H], BF16, kind="ExternalInput")
    wo = nc.dram_tensor("wo", [NHC, P, D], BF16, kind="ExternalInput")
    ropeq = nc.dram_tensor("ropeq", [2, P, T], BF16, kind="ExternalInput")
    ropek = nc.dram_tensor("ropek", [2, P, T], BF16, kind="ExternalInput")
    msk = nc.dram_tensor("msk", [3, P, 2 * P], F32, kind="ExternalInput")
    out = nc.dram_tensor("out", [TPC, D], F32, kind="ExternalOutput")

    with tile.TileContext(nc) as tc:
        with (
            tc.tile_pool(name="dram", bufs=1, space="DRAM") as dram,
            tc.tile_pool(name="consts", bufs=1) as consts,
            tc.tile_pool(name="qkv", bufs=1) as qkvpool,
            tc.tile_pool(name="attn_sb", bufs=2) as apool,
            tc.tile_pool(name="es_sb", bufs=3) as espool,
            tc.tile_pool(name="rb_sb", bufs=1) as rbpool,
        ):
            # A2A bounce buffers: [src_rank_block][local h chunk-major][tok]
            a2a_in = [dram.tile([NCORES, HLOC, CHTOK], BF16,
                                name=f"a2a_in{m}") for m in range(2)]
            a2a_out = [dram.tile([NCORES, HLOC, CHTOK], BF16,
                                 name=f"a2a_out{m}") for m in range(2)]

            ones_col = consts.tile([P, 1], BF16)
            nc.gpsimd.memset(ones_col, 1.0)
            msk_sb = consts.tile([P, 3, 2 * P], F32)

            qts, kts, vsbs = [], [], []
            P_POOLS = {}
            xT_v = xT[:].rearrange("(c p) t -> p c t", p=P)

            # weights/x pools close mid-schedule (after projections) so
            # their SBUF space is reused by the wo/ef pools.
            pstack = contextlib.ExitStack()
            wpool = pstack.enter_context(
                tc.tile_pool(name="weights", bufs=1))
            xpool = pstack.enter_context(tc.tile_pool(name="xt", bufs=2))
            rpool = pstack.enter_context(tc.tile_pool(name="rtmp", bufs=1))

            # ---- staged input loads: first slices small so the first
            # matmuls start within a few us of kernel start ----
            wq_sb = wpool.tile([P, DC, HLOC], BF16)
            wq_v = wq[:].rearrange("(c p) h -> p c h", p=P)
            nc.sync.dma_start(wq_sb[:, 0:4, :], wq_v[:, 0:4, :])
            xt00 = xpool.tile([P, DC, TPIECE], BF16, tag="xt")
            nc.sync.dma_start(xt00[:, 0:4, :], xT_v[:, 0:4, 0:TPIECE])
            for d0, d1 in ((4, 12), (12, 20), (20, DC)):
                nc.sync.dma_start(wq_sb[:, d0:d1, :], wq_v[:, d0:d1, :])
            nc.sync.dma_start(xt00[:, 4:, :], xT_v[:, 4:, 0:TPIECE])
            rq_sb = wpool.tile([P, 2, T], BF16)
            nc.sync.dma_start(rq_sb[:], ropeq[:].rearrange("s p t -> p s t"))
            wk_sb = wpool.tile([P, DC, H], BF16)
            nc.sync.dma_start(wk_sb[:],
                              wk[:].rearrange("(c p) h -> p c h", p=P))
            wv_sb = wpool.tile([P, DC, H], BF16)
            nc.sync.dma_start(wv_sb[:],
                              wv[:].rearrange("(c p) h -> p c h", p=P))
            rk_sb = wpool.tile([P, 2, T], BF16)
            nc.sync.dma_start(rk_sb[:], ropek[:].rearrange("s p t -> p s t"))
            nc.sync.dma_start(msk_sb[:], msk[:].rearrange("m p q -> p m q"))

            for b in range(B):
                qt = qkvpool.tile([P, 4, T], BF16, tag=f"qt{b}")
                kt = qkvpool.tile([P, 2, T], BF16, tag=f"kt{b}")
                vsb = qkvpool.tile([P, TQ, H], BF16, tag=f"v{b}")
                qts.append(qt)
                kts.append(kt)
                vsbs.append(vsb)

            # =================== projections + rope =======================
            # qt chunk order is (c*2 + h): [c0h0, c0h1, c1h0, c1h1] so the
            # QK matmul rhs for contraction chunk c is a contiguous slice.
            def proj_piece(b, pi):
                ppsum, vpsum = P_POOLS["ppsum"], P_POOLS["vpsum"]
                qt, kt, vsb = qts[b], kts[b], vsbs[b]
                t0 = pi * TPIECE
                if b == 0 and pi == 0:
                    xt = xt00
                else:
                    xt = xpool.tile([P, DC, TPIECE], BF16, tag="xt")
                    nc.sync.dma_start(
                        xt[:], xT_v[:, :, b * T + t0:b * T + t0 + TPIECE])

                def rope(dst, i0, i1, psA, psB, tab):
                    cos = tab[:, 0, t0:t0 + TPIECE]
                    sin = tab[:, 1, t0:t0 + TPIECE]
                    t1 = rpool.tile([P, TPIECE], F32, tag="r1")
                    t2 = rpool.tile([P, TPIECE], F32, tag="r2")
                    nc.vector.tensor_tensor(t1[:], psA[:], cos, ALU.mult)
                    nc.vector.tensor_tensor(t2[:], psB[:], sin, ALU.mult)
                    nc.vector.tensor_tensor(
                        dst[:, i0, t0:t0 + TPIECE], t1[:], t2[:],
                        ALU.subtract)
                    t3 = rpool.tile([P, TPIECE], F32, tag="r3")
                    t4 = rpool.tile([P, TPIECE], F32, tag="r4")
                    nc.vector.tensor_tensor(t3[:], psB[:], cos, ALU.mult)
                    nc.vector.tensor_tensor(t4[:], psA[:], sin, ALU.mult)
                    nc.vector.tensor_tensor(
                        dst[:, i1, t0:t0 + TPIECE], t3[:], t4[:], ALU.add)

                # Q projections: head h -> chunks h (c0) and 2+h (c1)
                for hh in range(2):
                    ps = []
                    for cc in range(2):
                        hc = cc * 2 + hh
                        pq = ppsum.tile([P, TPIECE], F32, tag="pq")
                        for dc in range(DC):
                            nc.tensor.matmul(
                                pq[:],
                                wq_sb[:, dc, hc * P:(hc + 1) * P],
                                xt[:, dc, :],
                                start=(dc == 0), stop=(dc == DC - 1))
                        ps.append(pq)
                    rope(qt, hh, 2 + hh, ps[0], ps[1], rq_sb)
                # K projection: 2 h-chunks
                ps = []
                for half in range(2):
                    pk = ppsum.tile([P, TPIECE], F32, tag="pq")
                    for dc in range(DC):
                        nc.tensor.matmul(
                            pk[:],
                            wk_sb[:, dc, half * P:(half + 1) * P],
                            xt[:, dc, :],
                            start=(dc == 0), stop=(dc == DC - 1))
                    ps.append(pk)
                rope(kt, 0, 1, ps[0], ps[1], rk_sb)
                # V projection: natural layout [t, h]
                for tc4 in range(TPIECE // P):
                    pv = vpsum.tile([P, H], F32, tag="pv")
                    for dc in range(DC):
                        nc.tensor.matmul(
                            pv[:],
                            xt[:, dc, tc4 * P:(tc4 + 1) * P],
                            wv_sb[:, dc, :],
                            start=(dc == 0), stop=(dc == DC - 1))
                    nc.vector.tensor_copy(
                        out=vsb[:, pi * (TPIECE // P) + tc4, :], in_=pv[:])

            # ======================= attention ===========================
            # Stage 1 issues QK + the softmax activations; stage 2 (run
            # later, with other PE work in between to hide the Scalar-
            # engine latency) does denominators, PV, normalize and ship.
            def attend_stage1(b, i):
                qkps = P_POOLS["qkps"]
                qt, kt = qts[b], kts[b]
                lo, hi = _band(i, mode)
                nj = hi - lo + 1
                nseg = (nj + SEGJ - 1) // SEGJ
                es_slices = []
                for si in range(nseg):
                    j0 = lo + si * SEGJ
                    j1 = min(j0 + SEGJ - 1, hi)
                    w = (j1 - j0 + 1) * 2 * P
                    ts_ = apool.tile([P, SEGJ * 2 * P], F32, tag=f"ts{si}")
                    # QK for pairs of key tiles sharing one PSUM bank; a
                    # single Tanh drains each bank into the SBUF strip.
                    jlist = list(range(j0, j1 + 1))
                    for pj in range(0, len(jlist), 2):
                        jpair = jlist[pj:pj + 2]
                        qk = qkps.tile([P, 4 * P], F32, tag="qk")
                        for u, j in enumerate(jpair):
                            for c in range(2):
                                nc.tensor.matmul(
                                    qk[:, u * 2 * P:(u + 1) * 2 * P],
                                    kt[:, c, j * P:(j + 1) * P],
                                    qt[:, 2 * c:2 * c + 2,
                                       i * P:(i + 1) * P],
                                    start=(c == 0), stop=(c == 1))
                        w2 = len(jpair) * 2 * P
                        nc.scalar.activation(
                            ts_[:, pj * 2 * P:pj * 2 * P + w2],
                            qk[:, :w2], AF.Tanh, scale=1.0 / SOFT_CAP)
                    for jj, j in enumerate(jlist):
                        mi = None
                        if j == i - WTILES:
                            mi = 0
                        elif j == i and mode == "tril":
                            mi = 1
                        elif j == i + WTILES and mode == "ones":
                            mi = 2
                        if mi is not None:
                            sl = slice(jj * 2 * P, (jj + 1) * 2 * P)
                            nc.vector.tensor_tensor(
                                ts_[:, sl], ts_[:, sl], msk_sb[:, mi, :],
                                ALU.add)
                    es = espool.tile([P, SEGJ * 2 * P], BF16, tag=f"es{si}")
                    nc.scalar.activation(es[:, :w], ts_[:, :w], AF.Exp,
                                         scale=SOFT_CAP)
                    for jj in range(len(jlist)):
                        es_slices.append(es[:, jj * 2 * P:(jj + 1) * 2 * P])
                return (b, i, lo, nj, es_slices)

            def attend_stage2(st):
                dnps, encps = P_POOLS["dnps"], P_POOLS["encps"]
                b, i, lo, nj, es_slices = st
                vsb = vsbs[b]
                # softmax denominators: dn[0, q] = sum_k es[k, q]
                dn = dnps.tile([1, 2 * P], F32, tag="dn")
                for jj, esl in enumerate(es_slices):
                    nc.tensor.matmul(dn[:], ones_col[:], esl,
                                     start=(jj == 0), stop=(jj == nj - 1))
                rinv = apool.tile([1, 2 * P], F32, tag="rinv")
                nc.vector.reciprocal_approx_fast(rinv[:], dn[:])
                rb = rbpool.tile([P, 2 * P], F32, tag="rb")
                nc.gpsimd.partition_broadcast(rb[:], rinv[:])
                # PV: encT chunks [hd_c, (h0|h1) q]; both c-chunks share one
                # PSUM bank. encsb is stored in a2a chunk order (h*2+c).
                enc = encps.tile([P, 4 * P], F32, tag="enc")
                encsb = apool.tile([P, 4, P], BF16, tag="encsb")
                encsb_v = encsb[:].rearrange("p (h c) t -> p c h t", h=2)
                for c in range(2):
                    for jj, esl in enumerate(es_slices):
                        j = lo + jj
                        nc.tensor.matmul(
                            enc[:, c * 2 * P:(c + 1) * 2 * P],
                            vsb[:, j, c * P:(c + 1) * P],
                            esl,
                            start=(jj == 0), stop=(jj == nj - 1))
                    nc.vector.tensor_tensor(
                        encsb_v[:, c],
                        enc[:, c * 2 * P:(c + 1) * 2 * P].rearrange(
                            "p (h t) -> p h t", h=2),
                        rb[:].rearrange("p (h t) -> p h t", h=2),
                        ALU.mult)
                gtok = b * T + i * P
                jb = gtok // TPC
                ch = (gtok % TPC) // CHTOK
                toff = gtok % CHTOK
                dst = a2a_in[ch][jb].rearrange(
                    "(c p) t -> p c t", p=P)[:, :, toff:toff + P]
                nc.sync.dma_start(dst, encsb[:])

            # ================ output projection helpers ===================
            wo_tiles = {}
            efs = {}
            O_POOLS = {}

            def load_wo(half, dp):
                t = O_POOLS["wo"].tile([P, NHC, DP], BF16, tag="wo")
                wv_ = wo[:, :, dp * DP:(dp + 1) * DP].rearrange(
                    "c p d -> p c d")
                for h0 in range(0, NHC, 8):
                    nc.gpsimd.dma_start(t[:, h0:h0 + 8, :],
                                        wv_[:, h0:h0 + 8, :])
                wo_tiles[(half, dp)] = t

            def load_ef(half):
                ef = O_POOLS["ef"].tile([P, NHC, CHTOK], BF16,
                                        tag=f"ef{half}")
                ev = a2a_out[half][:].rearrange("b (c p) t -> p (b c) t",
                                                p=P)
                for h0 in range(0, NHC, 8):
                    nc.gpsimd.dma_start(ef[:, h0:h0 + 8, :],
                                        ev[:, h0:h0 + 8, :])
                efs[half] = ef

            # ---- interleaved schedule ----
            # Every attend's stage 1 (QK + softmax issue) is emitted right
            # after the projection piece that completes its rows; stage 2
            # runs one piece later, when the softmax chain has certainly
            # drained. The b1 attends of A2A chunk 1 are held back to give
            # the PE work that covers A2A0 + the ef0/wo loads.
            ostack = contextlib.ExitStack()
            with (
                tc.tile_pool(name="proj_ps", bufs=3, space="PSUM") as ppsum,
                tc.tile_pool(name="projv_ps", bufs=1, space="PSUM") as vpsum,
                tc.tile_pool(name="qk_ps", bufs=2, space="PSUM") as qkps,
                tc.tile_pool(name="dn_ps", bufs=1, space="PSUM") as dnps,
                tc.tile_pool(name="enc_ps", bufs=1, space="PSUM") as encps,
            ):
                P_POOLS.update(ppsum=ppsum, vpsum=vpsum, qkps=qkps,
                               dnps=dnps, encps=encps)
                pend = []

                def flush():
                    while pend:
                        attend_stage2(pend.pop(0))

                for pi in range(NPIECE):
                    proj_piece(0, pi)
                    flush()
                    if pi % 2 == 0:       # rows 2pi, 2pi+1 are chunk 0
                        pend.append(attend_stage1(0, 2 * pi))
                        pend.append(attend_stage1(0, 2 * pi + 1))
                for pi in range(NPIECE):
                    proj_piece(1, pi)
                    flush()
                    if pi % 2 == 0:
                        pend.append(attend_stage1(1, 2 * pi))
                        pend.append(attend_stage1(1, 2 * pi + 1))
                flush()
                # projections done: free weights/x SBUF, open oproj pools
                pstack.close()
                O_POOLS["wo"] = ostack.enter_context(tc.tile_pool(
                    name="wo_sb", bufs=2 if mode == "tril" else 1))
                O_POOLS["ef"] = ostack.enter_context(
                    tc.tile_pool(name="ef", bufs=1))
                O_POOLS["osb"] = ostack.enter_context(
                    tc.tile_pool(name="osb", bufs=2))
                nc.gpsimd.collective_compute(
                    "AllToAll", ALU.bypass,
                    replica_groups=[list(range(NCORES))],
                    ins=[a2a_in[0][:].opt()], outs=[a2a_out[0][:].opt()])
                load_ef(0)
                load_wo(0, 0)
                load_wo(0, 1)
                for b in range(B):
                    for i in range(TQ):
                        if (i % 4) // 2 == 1:
                            while len(pend) >= 2:
                                attend_stage2(pend.pop(0))
                            pend.append(attend_stage1(b, i))
                flush()
                nc.gpsimd.collective_compute(
                    "AllToAll", ALU.bypass,
                    replica_groups=[list(range(NCORES))],
                    ins=[a2a_in[1][:].opt()], outs=[a2a_out[1][:].opt()])

            # ==================== output projection ====================
            with tc.tile_pool(name="oproj_ps", bufs=2,
                              space="PSUM") as opsum:
                for half in range(2):
                    if half == 1:
                        load_ef(1)
                    ef = efs[half]
                    for dp in range(D // DP):
                        if (half, dp) not in wo_tiles:
                            load_wo(half, dp)
                        wo_sb = wo_tiles[(half, dp)]
                        for t2 in range(2):
                            tc4 = half * 2 + t2
                            po = opsum.tile([P, DP], F32, tag="po")
                            for hc in range(NHC):
                                nc.tensor.matmul(
                                    po[:],
                                    ef[:, hc, t2 * P:(t2 + 1) * P],
                                    wo_sb[:, hc, :],
                                    start=(hc == 0), stop=(hc == NHC - 1))
                            osb = O_POOLS["osb"].tile([P, DP], F32,
                                                      tag="osb")
                            nc.scalar.activation(osb[:], po[:], AF.Copy)
                            nc.sync.dma_start(
                                out[tc4 * P:(tc4 + 1) * P,
                                    dp * DP:(dp + 1) * DP],
                                osb[:])
            ostack.close()

    nc.compile()
    return nc


def _rope_tables(pos, scale):
    """pos: [T] int array -> [2, 128, T] bf16 (cos;sin), scaled."""
    frac = 2.0 * np.arange(H // 2, dtype=np.float64) / H
    timescale = ROPE_BASE ** frac                      # [128]
    sinusoid = pos.astype(np.float64)[None, :] / timescale[:, None]  # [128,T]
    tabs = np.stack([np.cos(sinusoid), np.sin(sinusoid)]) * scale
    return tabs.astype(ml_dtypes.bfloat16)


def _masks():
    """[3, 128, 256] f32 additive masks in [k, q(2 heads)] layout."""
    kl = np.arange(P)[:, None]
    ql = np.arange(2 * P)[None, :] % P
    m0 = np.where(kl > ql, 0.0, MASKVAL)    # j == i-8: valid k_l > q_l
    m1 = np.where(kl <= ql, 0.0, MASKVAL)   # j == i (causal): valid k_l <= q_l
    m2 = np.where(kl < ql, 0.0, MASKVAL)    # j == i+8: valid k_l < q_l
    return np.stack([m0, m1, m2]).astype(np.float32)


def _reference_host(x, segment_pos, attn_mask, w_q, w_kv, w_o):
    """Slow but fully general fallback (numpy)."""
    xb = x.astype(np.float32)
    q = np.einsum('btd,ndh->btnh', xb, w_q)
    k = np.einsum('bsd,kdh->bskh', xb, w_kv[0])
    v = np.einsum('bsd,kdh->bskh', xb, w_kv[1])

    def rope(t, positions):
        hd = t.shape[-1]
        frac = 2.0 * np.arange(hd // 2, dtype=np.float32) / hd
        ts_ = ROPE_BASE ** frac
        sinusoid = positions.astype(np.float32)[..., None] / ts_
        sinusoid = sinusoid[..., None, :]
        s, c = np.sin(sinusoid), np.cos(sinusoid)
        first, second = np.split(t, 2, axis=-1)
        return np.concatenate([first * c - second * s,
                               second * c + first * s], axis=-1)

    q = rope(q, segment_pos) * SCALAR
    k = rope(k, segment_pos)
    qg = q.reshape(B, T, NKV, 2, H)
    logits = np.einsum('btkgh,bskh->btkgs', qg, k).reshape(B, T, NQ, T)
    logits = np.tanh(logits / SOFT_CAP) * SOFT_CAP
    pos_s = np.arange(T)[None, None, :]
    pos_t = segment_pos[:, :, None]
    sliding = (pos_s > pos_t - WINDOW) & (pos_s < pos_t + WINDOW)
    mask = np.logical_and(attn_mask, sliding)
    padded = np.where(mask[:, :, None, :], logits, -np.inf)
    padded -= padded.max(axis=-1, keepdims=True)
    e = np.exp(padded)
    probs = (e / e.sum(axis=-1, keepdims=True)).astype(np.float32)
    v_exp = np.repeat(v, NQ // NKV, axis=2)            # [B,T,NQ,H]
    enc = np.einsum('btns,bsnh->btnh', probs, v_exp)
    return np.einsum('btnh,nhd->btd', enc, w_o).astype(np.float32)


_GRAPH_CACHE = {}


def kernel(x, segment_pos, attn_mask, w_q, w_kv, w_o):
    global last_result
    x = np.asarray(x)
    segment_pos = np.asarray(segment_pos)
    attn_mask = np.asarray(attn_mask)
    w_q = np.asarray(w_q, dtype=np.float32)
    w_kv = np.asarray(w_kv, dtype=np.float32)
    w_o = np.asarray(w_o, dtype=np.float32)

    arange = np.broadcast_to(np.arange(T, dtype=segment_pos.dtype), (B, T))
    std_pos = np.array_equal(segment_pos, arange)
    tril = np.broadcast_to(np.tril(np.ones((T, T), dtype=bool)), (B, T, T))
    if attn_mask.all():
        mode = "ones"
    elif np.array_equal(attn_mask, tril):
        mode = "tril"
    else:
        mode = None
    if not std_pos or mode is None:
        return _reference_host(x, segment_pos, attn_mask, w_q, w_kv, w_o)

    if mode not in _GRAPH_CACHE:
        _GRAPH_CACHE[mode] = build(mode)
    nc = _GRAPH_CACHE[mode]

    bf = ml_dtypes.bfloat16
    xT = np.ascontiguousarray(x.reshape(TOK, D).T).astype(bf)    # [D, TOK]
    pos = segment_pos[0]
    ropeq = np.ascontiguousarray(_rope_tables(pos, SCALAR))
    ropek = np.ascontiguousarray(_rope_tables(pos, 1.0))
    wo_all = np.ascontiguousarray(
        w_o.reshape(NHC, P, D)).astype(bf)
    msk = np.ascontiguousarray(_masks())

    in_maps = []
    for c in range(NCORES):
        # wq columns in qt chunk order (c*2 + h): [c0h0, c0h1, c1h0, c1h1]
        g0, g1 = w_q[2 * c], w_q[2 * c + 1]
        wq_c = np.ascontiguousarray(
            np.concatenate([g0[:, :P], g1[:, :P], g0[:, P:], g1[:, P:]],
                           axis=1)).astype(bf)
        wk_c = np.ascontiguousarray(w_kv[0, c]).astype(bf)
        wv_c = np.ascontiguousarray(w_kv[1, c]).astype(bf)
        in_maps.append({
            "xT": xT, "wq": wq_c, "wk": wk_c, "wv": wv_c, "wo": wo_all,
            "ropeq": ropeq, "ropek": ropek, "msk": msk,
        })

    trace = os.environ.get("KTRACE", "0") == "1"
    res = run_bass_kernel_spmd(nc, in_maps, core_ids=list(range(NCORES)),
                               trace=trace)
    last_result = res
    outs = [res.results[c]["out"] for c in range(NCORES)]
    return np.concatenate(outs, axis=0).reshape(B, T, D).astype(np.float32)


# revision 35
# speedup vs baseline: 1.2493x; 1.0236x over previous
"""Sliding-window GQA attention (Gemma-style) on 8 TRN2 NeuronCores.

Sharding: tensor-parallel over heads. Core c owns q-heads {2c, 2c+1} and
kv-head c. Each core computes Q/K/V projections (+RoPE) for its heads over
the full sequence, banded sliding-window attention, then an AllToAll
(split into 2 token-chunks) reshards the attention output by token so
every core computes the full output projection for its 512-token slice.
Host concatenates slices.

Attention uses a [k, q] logits layout (K stationary, Q moving) so the
softmax probabilities leave the Scalar engine directly in the [k, q]
orientation PV needs -- no PE transposes. QK logits for two adjacent key
tiles share one PSUM bank so one Tanh activation drains them straight
into a wide SBUF strip; one Exp activation per band segment produces
bf16 probabilities. Window/causal masks are added by GpSimd. Softmax
denominators come from ones-vector matmuls accumulated in PSUM and the
normalization is applied once on the (much smaller) attention output.

Batch-0 attention is interleaved into batch-1's projection so the
Scalar/Vector-engine softmax work hides under projection matmuls.

All matmuls run in bf16 (f32 PSUM accumulation); softmax runs in f32.
"""

import contextlib
import os
import sys

for _p in ("/opt/trn_rl_repo",):
    if _p not in sys.path:
        sys.path.insert(0, _p)

import numpy as np
import ml_dtypes

import concourse.bass as bass
import concourse.mybir as mybir
import concourse.tile as tile
from concourse import bacc
from concourse.bass_utils import run_bass_kernel_spmd

F32 = mybir.dt.float32
BF16 = mybir.dt.bfloat16
AF = mybir.ActivationFunctionType
ALU = mybir.AluOpType

B, T, D = 2, 2048, 3584
NQ, NKV, H = 16, 8, 256
SCALAR = 0.0625
SOFT_CAP = 50.0
WINDOW = 1024
ROPE_BASE = 10000.0

NCORES = 8
P = 128
DC = D // P              # 28 contraction chunks
TQ = T // P              # 16 query tiles per batch
TPIECE = 256             # projection output tile width
NPIECE = T // TPIECE
HLOC = 2 * H             # 512 local q-head columns per core
TOK = B * T              # 4096
TPC = TOK // NCORES      # 512 tokens per core after AllToAll
CHTOK = TPC // 2         # 256 tokens per A2A chunk block
WTILES = WINDOW // P     # 8
MASKVAL = -1.0e30        # added to tanh output; exp(50*(t+MASKVAL)) == 0
NHC = NQ * H // P        # 32 global h chunks
DP = 512                 # output projection d piece
SEGJ = 9                 # max band tiles per softmax strip

last_result = None       # BassKernelResults of the most recent device run


def _band(i, mode):
    lo = max(0, i - WTILES)
    hi = i if mode == "tril" else min(TQ - 1, i + WTILES)
    return lo, hi


def build(mode):
    assert mode in ("tril", "ones")
    nc = bacc.Bacc("TRN2", target_bir_lowering=False, debug=False,
                   num_devices=NCORES)

    xT = nc.dram_tensor("xT", [D, TOK], BF16, kind="ExternalInput")
    wq = nc.dram_tensor("wq", [D, HLOC], BF16, kind="ExternalInput")
    wk = nc.dram_tensor("wk", [D, H], BF16, kind="ExternalInput")
    wv = nc.dram_tensor("wv", [D, H], BF16, kind="ExternalInput")
    wo = nc.dram_tensor("wo", [NHC, P, D], BF16, kind="ExternalInput")
    ropeq = nc.dram_tensor("ropeq", [2, P, T], BF16, kind="ExternalInput")
    ropek = nc.dram_tensor("ropek", [2, P, T], BF16, kind="ExternalInput")
    msk = nc.dram_tensor("msk", [3, P, 2 * P], F32, kind="ExternalInput")
    out = nc.dram_tensor("out", [TPC, D], F32, kind="ExternalOutput")

    with tile.TileContext(nc) as tc:
        with (
            tc.tile_pool(name="dram", bufs=1, space="DRAM") as dram,
            tc.tile_pool(name="consts", bufs=1) as consts,
            tc.tile_pool(name="qkv", bufs=1) as qkvpool,
            tc.tile_pool(name="attn_sb", bufs=2) as apool,
            tc.tile_pool(name="es_sb", bufs=3) as espool,
            tc.tile_pool(name="rb_sb", bufs=1) as rbpool,
        ):
            # A2A bounce buffers: [src_rank_block][local h chunk-major][tok]
            a2a_in = [dram.tile([NCORES, HLOC, CHTOK], BF16,
                                name=f"a2a_in{m}") for m in range(2)]
            a2a_out = [dram.tile([NCORES, HLOC, CHTOK], BF16,
                                 name=f"a2a_out{m}") for m in range(2)]

            ones_col = consts.tile([P, 1], BF16)
            nc.gpsimd.memset(ones_col, 1.0)
            msk_sb = consts.tile([P, 3, 2 * P], F32)

            qts, kts, vsbs = [], [], []
            P_POOLS = {}
            xT_v = xT[:].rearrange("(c p) t -> p c t", p=P)

            # weights/x pools close mid-schedule (after projections) so
            # their SBUF space is reused by the wo/ef pools.
            pstack = contextlib.ExitStack()
            wpool = pstack.enter_context(
                tc.tile_pool(name="weights", bufs=1))
            xpool = pstack.enter_context(tc.tile_pool(name="xt", bufs=2))
            rpool = pstack.enter_context(tc.tile_pool(name="rtmp", bufs=1))

            # ---- staged input loads: first slices small so the first
            # matmuls start within a few us of kernel start ----
            wq_sb = wpool.tile([P, DC, HLOC], BF16)
            wq_v = wq[:].rearrange("(c p) h -> p c h", p=P)
            nc.sync.dma_start(wq_sb[:, 0:4, :], wq_v[:, 0:4, :])
            xt00 = xpool.tile([P, DC, TPIECE], BF16, tag="xt")
            nc.sync.dma_start(xt00[:, 0:4, :], xT_v[:, 0:4, 0:TPIECE])
            for d0, d1 in ((4, 12), (12, 20), (20, DC)):
                nc.sync.dma_start(wq_sb[:, d0:d1, :], wq_v[:, d0:d1, :])
            nc.sync.dma_start(xt00[:, 4:, :], xT_v[:, 4:, 0:TPIECE])
            rq_sb = wpool.tile([P, 2, T], BF16)
            nc.sync.dma_start(rq_sb[:], ropeq[:].rearrange("s p t -> p s t"))
            wk_sb = wpool.tile([P, DC, H], BF16)
            nc.sync.dma_start(wk_sb[:],
                              wk[:].rearrange("(c p) h -> p c h", p=P))
            wv_sb = wpool.tile([P, DC, H], BF16)
            nc.sync.dma_start(wv_sb[:],
                              wv[:].rearrange("(c p) h -> p c h", p=P))
            rk_sb = wpool.tile([P, 2, T], BF16)
            nc.sync.dma_start(rk_sb[:], ropek[:].rearrange("s p t -> p s t"))
            nc.sync.dma_start(msk_sb[:], msk[:].rearrange("m p q -> p m q"))

            for b in range(B):
                qt = qkvpool.tile([P, 4, T], BF16, tag=f"qt{b}")
                kt = qkvpool.tile([P, 2, T], BF16, tag=f"kt{b}")
                vsb = qkvpool.tile([P, TQ, H], BF16, tag=f"v{b}")
                qts.append(qt)
                kts.append(kt)
                vsbs.append(vsb)

            # =================== projections + rope =======================
            # qt chunk order is (c*2 + h): [c0h0, c0h1, c1h0, c1h1] so the
            # QK matmul rhs for contraction chunk c is a contiguous slice.
            def proj_piece(b, pi):
                ppsum, vpsum = P_POOLS["ppsum"], P_POOLS["vpsum"]
                qt, kt, vsb = qts[b], kts[b], vsbs[b]
                t0 = pi * TPIECE
                if b == 0 and pi == 0:
                    xt = xt00
                else:
                    xt = xpool.tile([P, DC, TPIECE], BF16, tag="xt")
                    nc.sync.dma_start(
                        xt[:], xT_v[:, :, b * T + t0:b * T + t0 + TPIECE])

                def rope(dst, i0, i1, psA, psB, tab):
                    cos = tab[:, 0, t0:t0 + TPIECE]
                    sin = tab[:, 1, t0:t0 + TPIECE]
                    t1 = rpool.tile([P, TPIECE], F32, tag="r1")
                    t2 = rpool.tile([P, TPIECE], F32, tag="r2")
                    nc.vector.tensor_tensor(t1[:], psA[:], cos, ALU.mult)
                    nc.vector.tensor_tensor(t2[:], psB[:], sin, ALU.mult)
                    nc.vector.tensor_tensor(
                        dst[:, i0, t0:t0 + TPIECE], t1[:], t2[:],
                        ALU.subtract)
                    t3 = rpool.tile([P, TPIECE], F32, tag="r3")
                    t4 = rpool.tile([P, TPIECE], F32, tag="r4")
                    nc.vector.tensor_tensor(t3[:], psB[:], cos, ALU.mult)
                    nc.vector.tensor_tensor(t4[:], psA[:], sin, ALU.mult)
                    nc.vector.tensor_tensor(
                        dst[:, i1, t0:t0 + TPIECE], t3[:], t4[:], ALU.add)

                # Q projections: head h -> chunks h (c0) and 2+h (c1)
                for hh in range(2):
                    ps = []
                    for cc in range(2):
                        hc = cc * 2 + hh
                        pq = ppsum.tile([P, TPIECE], F32, tag="pq")
                        for dc in range(DC):
                            nc.tensor.matmul(
                                pq[:],
                                wq_sb[:, dc, hc * P:(hc + 1) * P],
                                xt[:, dc, :],
                                start=(dc == 0), stop=(dc == DC - 1))
                        ps.append(pq)
                    rope(qt, hh, 2 + hh, ps[0], ps[1], rq_sb)
                # K projection: 2 h-chunks
                ps = []
                for half in range(2):
                    pk = ppsum.tile([P, TPIECE], F32, tag="pq")
                    for dc in range(DC):
                        nc.tensor.matmul(
                            pk[:],
                            wk_sb[:, dc, half * P:(half + 1) * P],
                            xt[:, dc, :],
                            start=(dc == 0), stop=(dc == DC - 1))
                    ps.append(pk)
                rope(kt, 0, 1, ps[0], ps[1], rk_sb)
                # V projection: natural layout [t, h]
                for tc4 in range(TPIECE // P):
                    pv = vpsum.tile([P, H], F32, tag="pv")
                    for dc in range(DC):
                        nc.tensor.matmul(
                            pv[:],
                            xt[:, dc, tc4 * P:(tc4 + 1) * P],
                            wv_sb[:, dc, :],
                            start=(dc == 0), stop=(dc == DC - 1))
                    nc.vector.tensor_copy(
                        out=vsb[:, pi * (TPIECE // P) + tc4, :], in_=pv[:])

            # ======================= attention ===========================
            # Stage 1 issues QK + the softmax activations; stage 2 (run
            # later, with other PE work in between to hide the Scalar-
            # engine latency) does denominators, PV, normalize and ship.
            def attend_stage1(b, i):
                qkps = P_POOLS["qkps"]
                qt, kt = qts[b], kts[b]
                lo, hi = _band(i, mode)
                nj = hi - lo + 1
                nseg = (nj + SEGJ - 1) // SEGJ
                es_slices = []
                for si in range(nseg):
                    j0 = lo + si * SEGJ
                    j1 = min(j0 + SEGJ - 1, hi)
                    w = (j1 - j0 + 1) * 2 * P
                    ts_ = apool.tile([P, SEGJ * 2 * P], F32, tag=f"ts{si}")
                    # QK for pairs of key tiles sharing one PSUM bank; a
                    # single Tanh drains each bank into the SBUF strip.
                    jlist = list(range(j0, j1 + 1))
                    for pj in range(0, len(jlist), 2):
                        jpair = jlist[pj:pj + 2]
                        qk = qkps.tile([P, 4 * P], F32, tag="qk")
                        for u, j in enumerate(jpair):
                            for c in range(2):
                                nc.tensor.matmul(
                                    qk[:, u * 2 * P:(u + 1) * 2 * P],
                                    kt[:, c, j * P:(j + 1) * P],
                                    qt[:, 2 * c:2 * c + 2,
                                       i * P:(i + 1) * P],
                                    start=(c == 0), stop=(c == 1))
                        w2 = len(jpair) * 2 * P
                        nc.scalar.activation(
                            ts_[:, pj * 2 * P:pj * 2 * P + w2],
                            qk[:, :w2], AF.Tanh, scale=1.0 / SOFT_CAP)
                    for jj, j in enumerate(jlist):
                        mi = None
                        if j == i - WTILES:
                            mi = 0
                        elif j == i and mode == "tril":
                            mi = 1
                        elif j == i + WTILES and mode == "ones":
                            mi = 2
                        if mi is not None:
                            sl = slice(jj * 2 * P, (jj + 1) * 2 * P)
                            nc.vector.tensor_tensor(
                                ts_[:, sl], ts_[:, sl], msk_sb[:, mi, :],
                                ALU.add)
                    es = espool.tile([P, SEGJ * 2 * P], BF16, tag=f"es{si}")
                    nc.scalar.activation(es[:, :w], ts_[:, :w], AF.Exp,
                                         scale=SOFT_CAP)
                    for jj in range(len(jlist)):
                        es_slices.append(es[:, jj * 2 * P:(jj + 1) * 2 * P])
                return (b, i, lo, nj, es_slices)

            def attend_stage2(st):
                dnps, encps = P_POOLS["dnps"], P_POOLS["encps"]
                b, i, lo, nj, es_slices = st
                vsb = vsbs[b]
                # softmax denominators: dn[0, q] = sum_k es[k, q]
                dn = dnps.tile([1, 2 * P], F32, tag="dn")
                for jj, esl in enumerate(es_slices):
                    nc.tensor.matmul(dn[:], ones_col[:], esl,
                                     start=(jj == 0), stop=(jj == nj - 1))
                rinv = apool.tile([1, 2 * P], F32, tag="rinv")
                nc.vector.reciprocal_approx_fast(rinv[:], dn[:])
                rb = rbpool.tile([P, 2 * P], F32, tag="rb")
                nc.gpsimd.partition_broadcast(rb[:], rinv[:])
                # PV: encT chunks [hd_c, (h0|h1) q]; both c-chunks share one
                # PSUM bank. encsb is stored in a2a chunk order (h*2+c).
                enc = encps.tile([P, 4 * P], F32, tag="enc")
                encsb = apool.tile([P, 4, P], BF16, tag="encsb")
                encsb_v = encsb[:].rearrange("p (h c) t -> p c h t", h=2)
                for c in range(2):
                    for jj, esl in enumerate(es_slices):
                        j = lo + jj
                        nc.tensor.matmul(
                            enc[:, c * 2 * P:(c + 1) * 2 * P],
                            vsb[:, j, c * P:(c + 1) * P],
                            esl,
                            start=(jj == 0), stop=(jj == nj - 1))
                    nc.vector.tensor_tensor(
                        encsb_v[:, c],
                        enc[:, c * 2 * P:(c + 1) * 2 * P].rearrange(
                            "p (h t) -> p h t", h=2),
                        rb[:].rearrange("p (h t) -> p h t", h=2),
                        ALU.mult)
                # token->core map: core (i % 8) owns row-tile i of both
                # batches; chunk = batch, slot = i // 8.
                jb = i % NCORES
                toff = (i // NCORES) * P
                dst = a2a_in[b][jb].rearrange(
                    "(c p) t -> p c t", p=P)[:, :, toff:toff + P]
                nc.sync.dma_start(dst, encsb[:])

            # ================ output projection helpers ===================
            wo_tiles = {}
            efs = {}
            O_POOLS = {}

            def load_wo(half, dp):
                t = O_POOLS["wo"].tile([P, NHC, DP], BF16, tag="wo")
                wv_ = wo[:, :, dp * DP:(dp + 1) * DP].rearrange(
                    "c p d -> p c d")
                for h0 in range(0, NHC, 8):
                    nc.gpsimd.dma_start(t[:, h0:h0 + 8, :],
                                        wv_[:, h0:h0 + 8, :])
                wo_tiles[(half, dp)] = t

            def load_ef(half):
                ef = O_POOLS[f"ef{half}"].tile([P, NHC, CHTOK], BF16,
                                               tag=f"ef{half}")
                ev = a2a_out[half][:].rearrange("b (c p) t -> p (b c) t",
                                                p=P)
                for h0 in range(0, NHC, 8):
                    nc.gpsimd.dma_start(ef[:, h0:h0 + 8, :],
                                        ev[:, h0:h0 + 8, :])
                efs[half] = ef

            # ---- interleaved schedule ----
            # Chunk 0 of the AllToAll is all of batch 0: it fires early in
            # batch 1's projection and its transfer hides completely.
            # Chunk 1 (batch 1) is covered by output-projection pass A.
            ostack = contextlib.ExitStack()
            with (
                tc.tile_pool(name="proj_ps", bufs=3, space="PSUM") as ppsum,
                tc.tile_pool(name="projv_ps", bufs=1, space="PSUM") as vpsum,
                tc.tile_pool(name="qk_ps", bufs=2, space="PSUM") as qkps,
                tc.tile_pool(name="dn_ps", bufs=1, space="PSUM") as dnps,
                tc.tile_pool(name="enc_ps", bufs=1, space="PSUM") as encps,
            ):
                P_POOLS.update(ppsum=ppsum, vpsum=vpsum, qkps=qkps,
                               dnps=dnps, encps=encps)
                pend = []

                def flush():
                    while pend:
                        attend_stage2(pend.pop(0))

                def rows_ready(p):
                    lastj = 2 * p + 1
                    prevj = 2 * p - 1
                    return [r for r in range(TQ)
                            if prevj < _band(r, mode)[1] <= lastj]

                for b in range(B):
                    for pi in range(NPIECE):
                        proj_piece(b, pi)
                        flush()
                        for r in rows_ready(pi):
                            pend.append(attend_stage1(b, r))
                        if b == 1 and pi == 0:
                            # all batch-0 attends have shipped
                            nc.gpsimd.collective_compute(
                                "AllToAll", ALU.bypass,
                                replica_groups=[list(range(NCORES))],
                                ins=[a2a_in[0][:].opt()],
                                outs=[a2a_out[0][:].opt()])
                    # rows whose band extends past the last piece (ones)
                    for r in range(TQ):
                        if _band(r, mode)[1] > 2 * (NPIECE - 1) + 1:
                            while len(pend) >= 2:
                                attend_stage2(pend.pop(0))
                            pend.append(attend_stage1(b, r))
                    if b == 1:
                        flush()
                # projections done: free weights/x SBUF, open oproj pools
                pstack.close()
                O_POOLS["wo"] = ostack.enter_context(tc.tile_pool(
                    name="wo_sb", bufs=2 if mode == "tril" else 1))
                O_POOLS["ef0"] = ostack.enter_context(
                    tc.tile_pool(name="ef0", bufs=1))
                O_POOLS["ef1"] = ostack.enter_context(
                    tc.tile_pool(name="ef1", bufs=1))
                O_POOLS["osb"] = ostack.enter_context(
                    tc.tile_pool(name="osb", bufs=2))
                load_ef(0)
                load_wo(0, 0)
                load_wo(0, 1)
                nc.gpsimd.collective_compute(
                    "AllToAll", ALU.bypass,
                    replica_groups=[list(range(NCORES))],
                    ins=[a2a_in[1][:].opt()], outs=[a2a_out[1][:].opt()])

            # ==================== output projection ====================
            with tc.tile_pool(name="oproj_ps", bufs=2,
                              space="PSUM") as opsum:
                for half in range(2):
                    if half == 1:
                        load_ef(1)
                    ef = efs[half]
                    for dp in range(D // DP):
                        if (half, dp) not in wo_tiles:
                            load_wo(half, dp)
                        wo_sb = wo_tiles[(half, dp)]
                        for t2 in range(2):
                            tc4 = half * 2 + t2
                            po = opsum.tile([P, DP], F32, tag="po")
                            for hc in range(NHC):
                                nc.tensor.matmul(
                                    po[:],
                                    ef[:, hc, t2 * P:(t2 + 1) * P],
                                    wo_sb[:, hc, :],
                                    start=(hc == 0), stop=(hc == NHC - 1))
                            osb = O_POOLS["osb"].tile([P, DP], F32,
                                                      tag="osb")
                            nc.scalar.activation(osb[:], po[:], AF.Copy)
                            nc.sync.dma_start(
                                out[tc4 * P:(tc4 + 1) * P,
                                    dp * DP:(dp + 1) * DP],
                                osb[:])
            ostack.close()

    nc.compile()
    return nc


def _rope_tables(pos, scale):
    """pos: [T] int array -> [2, 128, T] bf16 (cos;sin), scaled."""
    frac = 2.0 * np.arange(H // 2, dtype=np.float64) / H
    timescale = ROPE_BASE ** frac                      # [128]
    sinusoid = pos.astype(np.float64)[None, :] / timescale[:, None]  # [128,T]
    tabs = np.stack([np.cos(sinusoid), np.sin(sinusoid)]) * scale
    return tabs.astype(ml_dtypes.bfloat16)


def _masks():
    """[3, 128, 256] f32 additive masks in [k, q(2 heads)] layout."""
    kl = np.arange(P)[:, None]
    ql = np.arange(2 * P)[None, :] % P
    m0 = np.where(kl > ql, 0.0, MASKVAL)    # j == i-8: valid k_l > q_l
    m1 = np.where(kl <= ql, 0.0, MASKVAL)   # j == i (causal): valid k_l <= q_l
    m2 = np.where(kl < ql, 0.0, MASKVAL)    # j == i+8: valid k_l < q_l
    return np.stack([m0, m1, m2]).astype(np.float32)


def _reference_host(x, segment_pos, attn_mask, w_q, w_kv, w_o):
    """Slow but fully general fallback (numpy)."""
    xb = x.astype(np.float32)
    q = np.einsum('btd,ndh->btnh', xb, w_q)
    k = np.einsum('bsd,kdh->bskh', xb, w_kv[0])
    v = np.einsum('bsd,kdh->bskh', xb, w_kv[1])

    def rope(t, positions):
        hd = t.shape[-1]
        frac = 2.0 * np.arange(hd // 2, dtype=np.float32) / hd
        ts_ = ROPE_BASE ** frac
        sinusoid = positions.astype(np.float32)[..., None] / ts_
        sinusoid = sinusoid[..., None, :]
        s, c = np.sin(sinusoid), np.cos(sinusoid)
        first, second = np.split(t, 2, axis=-1)
        return np.concatenate([first * c - second * s,
                               second * c + first * s], axis=-1)

    q = rope(q, segment_pos) * SCALAR
    k = rope(k, segment_pos)
    qg = q.reshape(B, T, NKV, 2, H)
    logits = np.einsum('btkgh,bskh->btkgs', qg, k).reshape(B, T, NQ, T)
    logits = np.tanh(logits / SOFT_CAP) * SOFT_CAP
    pos_s = np.arange(T)[None, None, :]
    pos_t = segment_pos[:, :, None]
    sliding = (pos_s > pos_t - WINDOW) & (pos_s < pos_t + WINDOW)
    mask = np.logical_and(attn_mask, sliding)
    padded = np.where(mask[:, :, None, :], logits, -np.inf)
    padded -= padded.max(axis=-1, keepdims=True)
    e = np.exp(padded)
    probs = (e / e.sum(axis=-1, keepdims=True)).astype(np.float32)
    v_exp = np.repeat(v, NQ // NKV, axis=2)            # [B,T,NQ,H]
    enc = np.einsum('btns,bsnh->btnh', probs, v_exp)
    return np.einsum('btnh,nhd->btd', enc, w_o).astype(np.float32)


_GRAPH_CACHE = {}


def kernel(x, segment_pos, attn_mask, w_q, w_kv, w_o):
    global last_result
    x = np.asarray(x)
    segment_pos = np.asarray(segment_pos)
    attn_mask = np.asarray(attn_mask)
    w_q = np.asarray(w_q, dtype=np.float32)
    w_kv = np.asarray(w_kv, dtype=np.float32)
    w_o = np.asarray(w_o, dtype=np.float32)

    arange = np.broadcast_to(np.arange(T, dtype=segment_pos.dtype), (B, T))
    std_pos = np.array_equal(segment_pos, arange)
    tril = np.broadcast_to(np.tril(np.ones((T, T), dtype=bool)), (B, T, T))
    if attn_mask.all():
        mode = "ones"
    elif np.array_equal(attn_mask, tril):
        mode = "tril"
    else:
        mode = None
    if not std_pos or mode is None:
        return _reference_host(x, segment_pos, attn_mask, w_q, w_kv, w_o)

    if mode not in _GRAPH_CACHE:
        _GRAPH_CACHE[mode] = build(mode)
    nc = _GRAPH_CACHE[mode]

    bf = ml_dtypes.bfloat16
    xT = np.ascontiguousarray(x.reshape(TOK, D).T).astype(bf)    # [D, TOK]
    pos = segment_pos[0]
    ropeq = np.ascontiguousarray(_rope_tables(pos, SCALAR))
    ropek = np.ascontiguousarray(_rope_tables(pos, 1.0))
    wo_all = np.ascontiguousarray(
        w_o.reshape(NHC, P, D)).astype(bf)
    msk = np.ascontiguousarray(_masks())

    in_maps = []
    for c in range(NCORES):
        # wq columns in qt chunk order (c*2 + h): [c0h0, c0h1, c1h0, c1h1]
        g0, g1 = w_q[2 * c], w_q[2 * c + 1]
        wq_c = np.ascontiguousarray(
            np.concatenate([g0[:, :P], g1[:, :P], g0[:, P:], g1[:, P:]],
                           axis=1)).astype(bf)
        wk_c = np.ascontiguousarray(w_kv[0, c]).astype(bf)
        wv_c = np.ascontiguousarray(w_kv[1, c]).astype(bf)
        in_maps.append({
            "xT": xT, "wq": wq_c, "wk": wk_c, "wv": wv_c, "wo": wo_all,
            "ropeq": ropeq, "ropek": ropek, "msk": msk,
        })

    trace = os.environ.get("KTRACE", "0") == "1"
    res = run_bass_kernel_spmd(nc, in_maps, core_ids=list(range(NCORES)),
                               trace=trace)
    last_result = res
    # Core c holds rows [b0 i=c, b0 i=c+8, b1 i=c, b1 i=c+8] (128 each).
    full = np.empty((B, T, D), dtype=np.float32)
    for c in range(NCORES):
        oc = res.results[c]["out"]
        for b in range(B):
            full[b, c * P:(c + 1) * P] = oc[(2 * b) * P:(2 * b + 1) * P]
            full[b, (c + 8) * P:(c + 9) * P] = \
                oc[(2 * b + 1) * P:(2 * b + 2) * P]
    return full
